# revision 28
# baseline (speedup 1.0000x reference)
"""Cross-attention Trainium2 kernel (nn_CrossAttention, B=2, L=2048, D=1024,
Dctx=768, 16 heads x 64).

Sharding: 8 cores = 2 (batch) x 4 (head-groups of 4 heads). Each core computes
its batch's Q/K/V projections for its 4 heads, attention, and a partial output
projection; the host sums the head-group partials and adds b_o.

v2 design (CoreSim cost model):
- AV computed TRANSPOSED: per (half, head, q-block of 128), accumulate
  out_T[q128, 65] = sum_j ex_j[qb-chunk].T @ [ones|v_j] over the 16 key
  blocks, with ex as the (free-to-reload) stationary operand. Cost-model
  charge is out-free-size (65) per matmul, halving AV cost vs the
  untransposed form. Column 0 carries the softmax denominator.
- Per-qb normalize (reciprocal_approx_fast on a strided d-gather + one
  tensor_scalar_mul), then a PE transpose (identity matmul) back to
  [64, q] and a DVE copy into aT for the output projection.
- exp runs on ACT (scale=32; wq/bq pre-scaled by 1/256 on host so scores
  arrive as u/32) with a few tiles offloaded to 2-inst custom-DVE
  polynomial exp (p3(u/32)^32, rel err ~5e-4) to balance engines.
- ex tiles are bf16 (stationary operand; halves SBUF so a full head-half
  of 16 tiles stays live for the lagged transposed-AV pass).
- Tail split: the final pair's (s2,s3) out-projection contraction is
  emitted as two bf16 partials (outT + outT2) summed on host, so the p0
  half runs a window early instead of serializing in the tail.
"""
import numpy as np

import concourse.bass as bass
import concourse.tile as tile
from concourse import bacc, mybir, bass_utils

# ---- custom DVE exp ops (runtime registration, documented extension path) ---
import concourse.dve_ops as dve_ops
from concourse.dve_ops import DveOp, OPS, CUSTOM_DVE_SPECS, _SUB_OPCODE_FOR_NAME
from concourse.dve_spec import Spec, Src0, C0, C1, C2, One, lower, sq
from concourse.dve_uop import DveOpSpec

_t = Src0
_p3 = One + _t * (C0 + _t * (C1 + _t * C2))


def _ref_exp_a(in0, in1, c0, c1, c2):
    t = in0.astype(np.float32)
    p = 1.0 + t * (np.float32(c0) + t * (np.float32(c1) + t * np.float32(c2)))
    return (p * p).astype(np.float32)


def _ref_exp_b(in0, in1, c0, c1, c2):
    y = in0.astype(np.float32)
    y = y * y
    y = y * y
    y = y * y
    return (y * y).astype(np.float32)


def _register(name, spec):
    if name in _SUB_OPCODE_FOR_NAME:
        return next(o for o in OPS if o.name == name)
    row = dve_ops._CUSTOM_DVE_ROW_BASE + len(OPS)
    assert row < 0x20
    _SUB_OPCODE_FOR_NAME[name] = row
    shas = {}
    for ver in ("v3", "v4"):
        s = DveOpSpec(name=name, opcode=row, uops=lower(spec, ver=ver),
                      rd1_en=False)
        shas[ver] = s.sha(ver)
    op = DveOp(name, spec, subdim=False, uops_sha=shas)
    OPS.append(op)
    CUSTOM_DVE_SPECS[name] = spec
    return op


EXP_A = _register("ANT_EXP_P3SQ1", Spec(body=sq(_p3), reference=_ref_exp_a))
EXP_B = _register("ANT_SQ4", Spec(body=sq(sq(sq(sq(Src0)))), reference=_ref_exp_b))

# minimax-ish p3 for e^t on [-0.27, 0.27] (c0 normalized to 1; the global
# p(0)^32 factor cancels in softmax): c1, c2, c3
EC1, EC2, EC3 = 1.00005423, 0.50272472, 0.16640462

F32R = mybir.dt.float32r
F32 = mybir.dt.float32
BF16 = mybir.dt.bfloat16
EXP = mybir.ActivationFunctionType.Exp

# Problem shape (hardcoded per harness contract)
B, LQ, D = 2, 2048, 1024
DCTX = 768
NH, HD = 16, 64
SCALE = 1.0 / 8.0

GH = 4                # heads per core
VW = HD + 1           # 65: [ones | v] lane per (j, h)
VAW = GH * VW         # 260
KT_Q = D // 128       # 8
KT_C = DCTX // 128    # 6
NLK = LQ // 128       # 16 key blocks
NQB = 8               # q-blocks of 128 per half
HALF = 1024


DEBUG = False


def _build():
    nc = bacc.Bacc("TRN2", target_bir_lowering=False, debug=False,
                   enable_asserts=False, num_devices=8)

    xT_d = nc.dram_tensor("xT", (D, LQ), BF16, kind="ExternalInput").ap()
    cT_d = nc.dram_tensor("ctxT", (DCTX, LQ), BF16, kind="ExternalInput").ap()
    wq_d = nc.dram_tensor("wq", (D, 256), BF16, kind="ExternalInput").ap()
    wk_d = nc.dram_tensor("wk", (DCTX, 256), BF16, kind="ExternalInput").ap()
    wv_d = nc.dram_tensor("wv", (DCTX, 256), BF16, kind="ExternalInput").ap()
    wo_d = nc.dram_tensor("wo", (256, D), BF16, kind="ExternalInput").ap()
    bq_d = nc.dram_tensor("bq", (128, 2), F32, kind="ExternalInput").ap()
    bk_d = nc.dram_tensor("bk", (128, 2), F32, kind="ExternalInput").ap()
    bv_d = nc.dram_tensor("bv", (128, 256), F32, kind="ExternalInput").ap()
    id_d = nc.dram_tensor("ident", (128, 128), BF16, kind="ExternalInput").ap()
    vo_d = nc.dram_tensor("vones", (128, NLK * GH), BF16,
                          kind="ExternalInput").ap()
    out_d = nc.dram_tensor("outT", (D, LQ), BF16, kind="ExternalOutput").ap()
    out2_d = nc.dram_tensor("outT2", (D, LQ), BF16, kind="ExternalOutput").ap()

    with tile.TileContext(nc) as tc:
        with tc.tile_pool(name="w", bufs=1) as wp, \
             tc.tile_pool(name="xt", bufs=2) as xtp, \
             tc.tile_pool(name="ct", bufs=4) as ctp, \
             tc.tile_pool(name="act", bufs=1) as actp, \
             tc.tile_pool(name="expp", bufs=34) as expp, \
             tc.tile_pool(name="tmpp", bufs=2) as tmpp, \
             tc.tile_pool(name="attn", bufs=6) as attnp, \
             tc.tile_pool(name="rdp", bufs=4) as rdp, \
             tc.tile_pool(name="outp", bufs=6) as outp, \
             tc.tile_pool(name="stp", bufs=2, space="PSUM") as stp, \
             tc.tile_pool(name="avp", bufs=1, space="PSUM") as avp, \
             tc.tile_pool(name="scr", bufs=2, space="PSUM") as scr:

            # ---- earliest deps first: K path ----
            wk_t = wp.tile([128, KT_C * 256], BF16, tag="wk")
            nc.gpsimd.dma_start(wk_t[:].rearrange("p (kt m) -> p kt m", m=256),
                                wk_d.rearrange("(kt p) m -> p kt m", p=128))
            bk_t = wp.tile([128, 2], F32, tag="bk")
            nc.scalar.dma_start(bk_t[:], bk_d[:])

            # ---- persistent activation tiles ----
            qT = [actp.tile([128, LQ], F32R, tag=f"qT{p}", name=f"qT{p}")
                  for p in range(2)]
            kT = [actp.tile([128, LQ], F32R, tag=f"kT{p}", name=f"kT{p}")
                  for p in range(2)]
            v_t = actp.tile([128, NLK * VAW], BF16, tag="v")
            aT = [actp.tile([128, LQ], BF16, tag=f"aT{p}", name=f"aT{p}")
                  for p in range(2)]
            id_t = wp.tile([128, 128], BF16, tag="id")

            # warm the ACT exp table during the lead-in DMAs
            warm_t = tmpp.tile([128, 1], F32, tag="warm")
            nc.scalar.activation(warm_t[:], bk_t[:, 0:1], EXP, scale=1.0)

            # ones columns of v_t via one strided DMA
            nc.scalar.dma_start(
                v_t[:].rearrange("p (l w) -> p l w", w=VW)[:, :, 0:1],
                vo_d.rearrange("p (l o) -> p l o", o=1))

            wq_t = wp.tile([128, KT_Q * 256], BF16, tag="wq")
            bq_t = wp.tile([128, 2], F32, tag="bq")
            wv_t = wp.tile([128, KT_C * 256], BF16, tag="wv")
            bv_t = wp.tile([128, 256], F32, tag="bv")
            wo_t = wp.tile([128, 2 * D], BF16, tag="wo")
            ct_tiles = {}
            xt_tiles = {}

            def ct_dma(s, eng=None):
                eng = eng or nc.sync
                t = ctp.tile([128, KT_C * 512], BF16, tag="ct")
                tv = t[:].rearrange("p (kt q) -> p kt q", q=512)
                cv = cT_d.rearrange("(kt p) q -> p kt q",
                                    p=128)[:, :, 512 * s:512 * (s + 1)]
                for kk in range(3):
                    eng.dma_start(tv[:, 2 * kk:2 * kk + 2, :],
                                  cv[:, 2 * kk:2 * kk + 2, :])
                ct_tiles[s] = t

            def k_proj(s, p):
                if s not in ct_tiles:
                    ct_dma(s)
                t = ct_tiles[s]
                ps = scr.tile([128, 512], F32, tag="u")
                for kt in range(KT_C):
                    nc.tensor.matmul(
                        ps[:], wk_t[:, 256 * kt + 128 * p:256 * kt + 128 * (p + 1)],
                        t[:, 512 * kt:512 * (kt + 1)],
                        start=(kt == 0), stop=(kt == KT_C - 1))
                nc.vector.tensor_scalar_add(
                    kT[p][:, 512 * s:512 * (s + 1)], ps[:], bk_t[:, p:p + 1])

            def xt_dma(s):
                t = xtp.tile([128, KT_Q * 512], BF16, tag="xt")
                tv = t[:].rearrange("p (kt q) -> p kt q", q=512)
                xv = xT_d.rearrange("(kt p) q -> p kt q",
                                    p=128)[:, :, 512 * s:512 * (s + 1)]
                for kk in range(2):
                    nc.sync.dma_start(tv[:, 4 * kk:4 * kk + 4, :],
                                      xv[:, 4 * kk:4 * kk + 4, :])
                xt_tiles[s] = t

            def q_proj(s, p):
                if s not in xt_tiles:
                    xt_dma(s)
                t = xt_tiles[s]
                ps = scr.tile([128, 512], F32, tag="u")
                for kt in range(KT_Q):
                    nc.tensor.matmul(
                        ps[:], wq_t[:, 256 * kt + 128 * p:256 * kt + 128 * (p + 1)],
                        t[:, 512 * kt:512 * (kt + 1)],
                        start=(kt == 0), stop=(kt == KT_Q - 1))
                nc.vector.tensor_scalar_add(
                    qT[p][:, 512 * s:512 * (s + 1)], ps[:], bq_t[:, p:p + 1])

            def v_chunk(j):
                ps = scr.tile([128, 256], F32, tag="u")
                s, jj = j // 4, j % 4
                for kt in range(KT_C):
                    nc.tensor.matmul(
                        ps[:],
                        ct_tiles[s][:, 512 * kt + 128 * jj:512 * kt + 128 * (jj + 1)],
                        wv_t[:, 256 * kt:256 * (kt + 1)],
                        start=(kt == 0), stop=(kt == KT_C - 1))
                vv = v_t[:, VAW * j:VAW * (j + 1)].rearrange(
                    "p (h w) -> p h w", w=VW)[:, :, 1:VW]
                nc.vector.tensor_add(
                    vv, ps[:].rearrange("p (h w) -> p h w", w=HD),
                    bv_t[:].rearrange("p (h w) -> p h w", w=HD))

            # ---- per-(half, head) attention state ----
            avt = [avp.tile([128, 512], F32, tag=f"avt{i}", name=f"avt{i}")
                   for i in range(2)]

            def avt_run(ctx, qb):
                """One transposed-AV accumulation run for a retiring head."""
                exs, h_ = ctx["exs"], ctx["h"]
                ps = avt[qb // 4]
                off = VW * (qb % 4)
                for j in range(NLK):
                    nc.tensor.matmul(
                        ps[:, off:off + VW],
                        exs[j][:, 128 * qb:128 * (qb + 1)],
                        v_t[:, VAW * j + VW * h_:VAW * j + VW * (h_ + 1)],
                        start=(j == 0), stop=(j == NLK - 1))

            def avt_recip(ctx, bank):
                """1/d for the 4 qb blocks of one avt bank."""
                rd = rdp.tile([128, 4], F32, tag=f"rd{bank}")
                dg = avt[bank][:, 0:4 * VW:VW]
                nc.vector.reciprocal_approx_fast(rd[:], dg)
                ctx[f"rd{bank}"] = rd

            def avt_norm(ctx, bank):
                """normalize a whole avt bank -> attn_sb bf16 [128q, 4, 64]."""
                rd = ctx[f"rd{bank}"]
                at = attnp.tile([128, 4 * HD], BF16, tag="at")
                src = avt[bank][:, 0:4 * VW].rearrange(
                    "p (b w) -> p b w", w=VW)[:, :, 1:VW]
                rdb = rd[:].rearrange("p (b o) -> p b o", o=1).broadcast_to(
                    [128, 4, HD])
                nc.vector.tensor_mul(
                    at[:].rearrange("p (b w) -> p b w", w=HD), src, rdb)
                ctx[f"at{bank}"] = at

            def avt_tr(ctx, qb):
                """PE-transpose attn qb into the bank's tp psum tile."""
                half_, h_ = ctx["half"], ctx["h"]
                m_ = h_ % 2
                bank = qb // 4
                at = ctx[f"at{bank}"]
                if qb % 4 == 0:
                    tpn = scr.tile([128, 4 * 128], BF16, tag="u")
                    ctx[f"tp{bank}"] = tpn
                tp = ctx[f"tp{bank}"]
                nc.tensor.matmul(
                    tp[64 * m_:64 * (m_ + 1), 128 * (qb % 4):128 * (qb % 4 + 1)],
                    at[:, HD * (qb % 4):HD * (qb % 4 + 1)], id_t[:],
                    is_transpose=True)

            def avt_cp(ctx, bank):
                """copy a bank of transposed attn into aT."""
                half_, h_ = ctx["half"], ctx["h"]
                p_, m_ = h_ // 2, h_ % 2
                tp = ctx.pop(f"tp{bank}")
                nc.vector.tensor_copy(
                    aT[p_][64 * m_:64 * (m_ + 1),
                           HALF * half_ + 512 * bank:HALF * half_ + 512 * (bank + 1)],
                    tp[64 * m_:64 * (m_ + 1), :])

            def out_unit(mo, s, p_, on_act=False):
                """out-proj partial unit for pair p_: one MM + copy + DMA."""
                ps = scr.tile([128, 512], F32, tag="u")
                nc.tensor.matmul(
                    ps[:], wo_t[:, D * p_ + 128 * mo:D * p_ + 128 * (mo + 1)],
                    aT[p_][:, 512 * s:512 * (s + 1)],
                    start=True, stop=True)
                ot = outp.tile([128, 512], BF16, tag="out")
                if on_act:
                    nc.scalar.copy(ot[:], ps[:])
                else:
                    nc.vector.tensor_copy(ot[:], ps[:])
                dst = out_d if p_ == 0 else out2_d
                nc.gpsimd.dma_start(
                    dst[128 * mo:128 * (mo + 1), 512 * s:512 * (s + 1)], ot[:])

            # ---- lead-in: DMAs spread across engine queues ----
            nc.scalar.dma_start(wq_t[:].rearrange("p (kt m) -> p kt m", m=256),
                                wq_d.rearrange("(kt p) m -> p kt m", p=128))
            xt_dma(0)                       # SP
            ct_dma(0, nc.gpsimd)            # Pool (behind wk)
            nc.scalar.dma_start(bq_t[:], bq_d[:])
            xt_dma(1)                       # SP
            ct_dma(1, nc.gpsimd)
            nc.scalar.dma_start(id_t[:], id_d[:])
            nc.gpsimd.dma_start(wv_t[:].rearrange("p (kt m) -> p kt m", m=256),
                                wv_d.rearrange("(kt p) m -> p kt m", p=128))
            nc.scalar.dma_start(bv_t[:], bv_d[:])
            nc.gpsimd.dma_start(wo_t[:].rearrange("p (p2 m) -> p p2 m", m=1024),
                                wo_d.rearrange("(p2 p) m -> p p2 m", p=128))
            k_proj(0, 0)
            q_proj(0, 0)
            q_proj(1, 0)

            # ---- drip worklists per window (list of (pos, 0-arg callable)) --
            def W(fn, *a):
                return lambda: fn(*a)

            def retire_steps(ctx, t0, dt):
                """AV_T + recip + norm + transpose steps for a finished head,
                spread from emission position t0 with spacing dt."""
                st = []
                t = t0
                for b in range(2):
                    for qq in range(4):
                        st.append((t, W(avt_run, ctx, 4 * b + qq)))
                        t += dt
                    st.append((t, W(avt_recip, ctx, b)))
                    t += dt / 4
                for bank in range(2):
                    st.append((t, W(avt_norm, ctx, bank)))
                    t += dt / 3
                    for qq in range(4):
                        st.append((t, W(avt_tr, ctx, 4 * bank + qq)))
                        t += dt / 3
                    st.append((t, W(avt_cp, ctx, bank)))
                    t += dt / 3
                return st

            # DVE-offloaded exp tiles per window
            DVE_JS = {0, 1}

            def emit_exp(st, j, on_dve):
                ex = expp.tile([128, HALF], BF16, tag="expS")
                if on_dve:
                    stg = tmpp.tile([128, HALF], F32, tag="stg")
                    nc.scalar.copy(stg[:], st[:])
                    tmp = tmpp.tile([128, HALF], F32, tag="tmp")
                    nc.vector._custom_dve(EXP_A, out=tmp[:], in0=stg[:],
                                          s0=EC1, s1=EC2, imm2=EC3)
                    nc.vector._custom_dve(EXP_B, out=ex[:], in0=tmp[:])
                else:
                    nc.scalar.activation(ex[:], st[:], EXP, scale=32.0)
                return ex

            # static drips (beyond the retire pipeline), per window
            wl = [[] for _ in range(9)]
            wl[0] = [
                (1, W(ct_dma, 2)), (2, W(k_proj, 1, 0)), (3, W(v_chunk, 0)),
                (4, W(v_chunk, 1)), (4.5, W(ct_dma, 3)), (5, W(k_proj, 2, 0)),
                (6, W(v_chunk, 2)), (7, W(v_chunk, 3)), (8, W(v_chunk, 4)),
                (9, W(k_proj, 3, 0)), (10, W(v_chunk, 5)), (11, W(v_chunk, 6)),
                (12, W(v_chunk, 7)), (13, W(v_chunk, 8)), (14, W(v_chunk, 9)),
            ]
            wl[1] = [
                (0, W(v_chunk, 10)), (1, W(v_chunk, 11)), (2, W(v_chunk, 12)),
                (3, W(v_chunk, 13)), (4, W(v_chunk, 14)), (5, W(v_chunk, 15)),
                (5.5, W(k_proj, 0, 1)), (8, W(q_proj, 0, 1)),
                (11, W(q_proj, 1, 1)),
            ]
            wl[2] = [(1, W(k_proj, 1, 1)),
                     (6, W(k_proj, 2, 1)), (10, W(k_proj, 3, 1))]
            wl[3] = ([(2, W(q_proj, 2, 0)), (5, W(q_proj, 3, 0))]
                     + [(8 + m, W(out_unit, m, m % 2, 0)) for m in range(8)])
            wl[4] = ([(2, W(q_proj, 2, 1)), (5, W(q_proj, 3, 1))]
                     + [(8 + m, W(out_unit, m, 1 - m % 2, 0)) for m in range(8)])
            wl[5] = ([(4 + m, W(out_unit, m, m % 2, 1)) for m in range(8)]
                     + [(12.5 + m / 4, W(out_unit, m, 1 - m % 2, 1))
                        for m in range(4)])
            wl[6] = ([(4 + m, W(out_unit, m, 1 - m % 2, 1)) for m in range(4, 8)]
                     + [(9 + m, W(out_unit, m, 2 + m % 2, 0)) for m in range(4)])
            wl[7] = ([(2 + m, W(out_unit, m, 2 + m % 2, 0)) for m in range(4, 8)]
                     + [(8 + m / 2, W(out_unit, m, 3 - m % 2, 0))
                        for m in range(8)])
            wl[8] = []

            # ---- main windows ----
            ctx_prev = None
            for w in range(8):
                half, h = w // 4, w % 4
                p, m = h // 2, h % 2
                r0 = 64 * m
                work = list(wl[w])
                if ctx_prev is not None:
                    # w1: v_chunks land j0-5, retire after; else spread early
                    work += retire_steps(ctx_prev, 6.0 if w == 1 else 0.5, 0.6)
                work.sort(key=lambda t: t[0])
                wi = 0
                exs = []
                for j in range(NLK):
                    while wi < len(work) and work[wi][0] <= j:
                        work[wi][1]()
                        wi += 1
                    st = stp.tile([128, HALF], F32, tag="st")
                    for n in range(2):
                        nc.tensor.matmul(
                            st[:, 512 * n:512 * (n + 1)],
                            kT[p][r0:r0 + 64, 128 * j:128 * (j + 1)],
                            qT[p][r0:r0 + 64,
                                  HALF * half + 512 * n:HALF * half + 512 * (n + 1)],
                            start=True, stop=True)
                    exs.append(emit_exp(st, j, j in DVE_JS))
                while wi < len(work):
                    work[wi][1]()
                    wi += 1
                ctx_prev = {"exs": exs, "half": half, "h": h}

            # ---- tail: retire h3-half1, rest of p0 partials, p1 partials ----
            tail = wl[8] + retire_steps(ctx_prev, 0.0, 0.4)
            tail.sort(key=lambda t: t[0])
            for _, cb in tail:
                cb()
            for mo in range(8):
                out_unit(mo, 2, 1, on_act=True)
                out_unit(mo, 3, 1, on_act=(mo % 2 == 0))

            if DEBUG:
                dbg_a = nc.dram_tensor("dbg_aT0", (128, LQ), F32,
                                       kind="ExternalOutput").ap()
                dbg_q = nc.dram_tensor("dbg_qT0", (128, LQ), F32,
                                       kind="ExternalOutput").ap()
                dbg_k = nc.dram_tensor("dbg_kT0", (128, LQ), F32,
                                       kind="ExternalOutput").ap()
                dbg_a1 = nc.dram_tensor("dbg_aT1", (128, LQ), F32,
                                        kind="ExternalOutput").ap()
                for nm, dst, src in (("a", dbg_a, aT[0]), ("q", dbg_q, qT[0]),
                                     ("k", dbg_k, kT[0]), ("a1", dbg_a1, aT[1])):
                    for c in range(4):
                        t = outp.tile([128, 512], F32, tag="dbg")
                        nc.vector.tensor_copy(t[:], src[:, 512 * c:512 * (c + 1)])
                        nc.gpsimd.dma_start(dst[:, 512 * c:512 * (c + 1)], t[:])

    nc.compile()
    return nc


_NC_CACHE = []


def _get_nc():
    if not _NC_CACHE:
        _NC_CACHE.append(_build())
    return _NC_CACHE[0]


OUT_NAME = "outT"


def prep_maps(inputs):
    """Host-side prep: per-core input tensor maps."""
    import ml_dtypes
    bf16 = ml_dtypes.bfloat16
    x = np.asarray(inputs["x"], np.float32)
    context = np.asarray(inputs["context"], np.float32)
    w_q = np.asarray(inputs["w_q"], np.float32)
    b_q = np.asarray(inputs["b_q"], np.float32)
    w_k = np.asarray(inputs["w_k"], np.float32)
    b_k = np.asarray(inputs["b_k"], np.float32)
    w_v = np.asarray(inputs["w_v"], np.float32)
    b_v = np.asarray(inputs["b_v"], np.float32)
    w_o = np.asarray(inputs["w_o"], np.float32)

    xTb = [np.ascontiguousarray(x[b].T).astype(bf16) for b in range(B)]
    cTb = [np.ascontiguousarray(context[b].T).astype(bf16) for b in range(B)]
    ident = np.eye(128, dtype=np.float32).astype(bf16)
    vones = np.ones((128, NLK * GH), np.float32).astype(bf16)
    maps = []
    for c in range(8):
        b, g = c // 4, c % 4
        hs = slice(256 * g, 256 * (g + 1))
        maps.append({
            "xT": xTb[b],
            "ctxT": cTb[b],
            # scale by 2^-8 (exact in fp): exp scale 32 * score scale 1/8
            "wq": (np.ascontiguousarray(w_q[:, hs]) / 256.0).astype(bf16),
            "wk": np.ascontiguousarray(w_k[:, hs]).astype(bf16),
            "wv": np.ascontiguousarray(w_v[:, hs]).astype(bf16),
            "wo": np.ascontiguousarray(w_o[hs, :]).astype(bf16),
            "bq": np.ascontiguousarray((b_q[hs] / 256.0).reshape(2, 128).T),
            "bk": np.ascontiguousarray(b_k[hs].reshape(2, 128).T),
            "bv": np.broadcast_to(b_v[None, hs], (128, 256)).copy(),
            "ident": ident,
            "vones": vones,
        })
    return maps


def kernel_run(inputs, trace=False, **kw):
    """Run on HW; returns (full_output, BassKernelResults)."""
    b_o = np.asarray(inputs["b_o"], np.float32)
    maps = prep_maps(inputs)
    nc = _get_nc()
    res = bass_utils.run_bass_kernel_spmd(nc, maps, core_ids=list(range(8)),
                                          trace=trace, **kw)
    out = np.empty((B, LQ, D), np.float32)
    for b in range(B):
        acc = res.results[4 * b]["outT"].astype(np.float32)
        acc += res.results[4 * b]["outT2"]
        for g in range(1, 4):
            acc = acc + res.results[4 * b + g]["outT"]
            acc = acc + res.results[4 * b + g]["outT2"]
        out[b] = acc.T + b_o[None, :]
    return out, res


def kernel(**inputs) -> np.ndarray:
    out, _ = kernel_run(inputs)
    return out


# revision 29
# speedup vs baseline: 1.0922x; 1.0922x over previous
"""Cross-attention Trainium2 kernel (nn_CrossAttention, B=2, L=2048, D=1024,
Dctx=768, 16 heads x 64).

Sharding: 8 cores = 2 (batch) x 4 (head-groups of 4 heads). Each core computes
its batch's Q/K/V projections for its 4 heads, attention, and a partial output
projection; the host sums the head-group partials and adds b_o.

v2 design (CoreSim cost model):
- AV computed TRANSPOSED: per (half, head, q-block of 128), accumulate
  out_T[q128, 65] = sum_j ex_j[qb-chunk].T @ [ones|v_j] over the 16 key
  blocks, with ex as the (free-to-reload) stationary operand. Cost-model
  charge is out-free-size (65) per matmul, halving AV cost vs the
  untransposed form. Column 0 carries the softmax denominator.
- Per-qb normalize (reciprocal_approx_fast on a strided d-gather + one
  tensor_scalar_mul), then a PE transpose (identity matmul) back to
  [64, q] and a DVE copy into aT for the output projection.
- exp runs on ACT (scale=32; wq/bq pre-scaled by 1/256 on host so scores
  arrive as u/32) with a few tiles offloaded to 2-inst custom-DVE
  polynomial exp (p3(u/32)^32, rel err ~5e-4) to balance engines.
- ex tiles are bf16 (stationary operand; halves SBUF so a full head-half
  of 16 tiles stays live for the lagged transposed-AV pass).
- Tail split: the final pair's (s2,s3) out-projection contraction is
  emitted as two bf16 partials (outT + outT2) summed on host, so the p0
  half runs a window early instead of serializing in the tail.
"""
import numpy as np

import concourse.bass as bass
import concourse.tile as tile
from concourse import bacc, mybir, bass_utils

# ---- custom DVE exp ops (runtime registration, documented extension path) ---
import concourse.dve_ops as dve_ops
from concourse.dve_ops import DveOp, OPS, CUSTOM_DVE_SPECS, _SUB_OPCODE_FOR_NAME
from concourse.dve_spec import Spec, Src0, C0, C1, C2, One, lower, sq
from concourse.dve_uop import DveOpSpec

_t = Src0
_p3 = One + _t * (C0 + _t * (C1 + _t * C2))


def _ref_exp_a(in0, in1, c0, c1, c2):
    t = in0.astype(np.float32)
    p = 1.0 + t * (np.float32(c0) + t * (np.float32(c1) + t * np.float32(c2)))
    return (p * p).astype(np.float32)


def _ref_exp_b(in0, in1, c0, c1, c2):
    y = in0.astype(np.float32)
    y = y * y
    y = y * y
    y = y * y
    return (y * y).astype(np.float32)


def _register(name, spec):
    if name in _SUB_OPCODE_FOR_NAME:
        return next(o for o in OPS if o.name == name)
    row = dve_ops._CUSTOM_DVE_ROW_BASE + len(OPS)
    assert row < 0x20
    _SUB_OPCODE_FOR_NAME[name] = row
    shas = {}
    for ver in ("v3", "v4"):
        s = DveOpSpec(name=name, opcode=row, uops=lower(spec, ver=ver),
                      rd1_en=False)
        shas[ver] = s.sha(ver)
    op = DveOp(name, spec, subdim=False, uops_sha=shas)
    OPS.append(op)
    CUSTOM_DVE_SPECS[name] = spec
    return op


EXP_A = _register("ANT_EXP_P3SQ1", Spec(body=sq(_p3), reference=_ref_exp_a))
EXP_B = _register("ANT_SQ4", Spec(body=sq(sq(sq(sq(Src0)))), reference=_ref_exp_b))

# minimax-ish p3 for e^t on [-0.27, 0.27] (c0 normalized to 1; the global
# p(0)^32 factor cancels in softmax): c1, c2, c3
EC1, EC2, EC3 = 1.00005423, 0.50272472, 0.16640462

F32R = mybir.dt.float32r
F32 = mybir.dt.float32
BF16 = mybir.dt.bfloat16
EXP = mybir.ActivationFunctionType.Exp

# Problem shape (hardcoded per harness contract)
B, LQ, D = 2, 2048, 1024
DCTX = 768
NH, HD = 16, 64
SCALE = 1.0 / 8.0

GH = 4                # heads per core
VW = HD + 1           # 65: [ones | v] lane per (j, h)
VAW = GH * VW         # 260
KT_Q = D // 128       # 8
KT_C = DCTX // 128    # 6
NLK = LQ // 128       # 16 key blocks
NQB = 8               # q-blocks of 128 per half
HALF = 1024


DEBUG = False


def _build():
    nc = bacc.Bacc("TRN2", target_bir_lowering=False, debug=False,
                   enable_asserts=False, num_devices=8)

    xT_d = nc.dram_tensor("xT", (D, LQ), BF16, kind="ExternalInput").ap()
    cT_d = nc.dram_tensor("ctxT", (DCTX, LQ), BF16, kind="ExternalInput").ap()
    wq_d = nc.dram_tensor("wq", (D, 256), BF16, kind="ExternalInput").ap()
    wk_d = nc.dram_tensor("wk", (DCTX, 256), BF16, kind="ExternalInput").ap()
    wv_d = nc.dram_tensor("wv", (DCTX, 256), BF16, kind="ExternalInput").ap()
    wo_d = nc.dram_tensor("wo", (256, D), BF16, kind="ExternalInput").ap()
    bq_d = nc.dram_tensor("bq", (128, 2), F32, kind="ExternalInput").ap()
    bk_d = nc.dram_tensor("bk", (128, 2), F32, kind="ExternalInput").ap()
    bv_d = nc.dram_tensor("bv", (128, 256), F32, kind="ExternalInput").ap()
    id_d = nc.dram_tensor("ident", (128, 128), BF16, kind="ExternalInput").ap()
    vo_d = nc.dram_tensor("vones", (128, NLK * GH), BF16,
                          kind="ExternalInput").ap()
    out_d = nc.dram_tensor("outT", (D, LQ), BF16, kind="ExternalOutput").ap()
    out2_d = nc.dram_tensor("outT2", (D, LQ), BF16, kind="ExternalOutput").ap()

    with tile.TileContext(nc) as tc:
        with tc.tile_pool(name="w", bufs=1) as wp, \
             tc.tile_pool(name="xt", bufs=2) as xtp, \
             tc.tile_pool(name="ct", bufs=4) as ctp, \
             tc.tile_pool(name="act", bufs=1) as actp, \
             tc.tile_pool(name="expp", bufs=34) as expp, \
             tc.tile_pool(name="tmpp", bufs=2) as tmpp, \
             tc.tile_pool(name="attn", bufs=6) as attnp, \
             tc.tile_pool(name="rdp", bufs=4) as rdp, \
             tc.tile_pool(name="outp", bufs=6) as outp, \
             tc.tile_pool(name="stp", bufs=2, space="PSUM") as stp, \
             tc.tile_pool(name="avp", bufs=1, space="PSUM") as avp, \
             tc.tile_pool(name="scr", bufs=2, space="PSUM") as scr:

            # ---- earliest deps first: K path ----
            wk_t = wp.tile([128, KT_C * 256], BF16, tag="wk")
            nc.gpsimd.dma_start(wk_t[:].rearrange("p (kt m) -> p kt m", m=256),
                                wk_d.rearrange("(kt p) m -> p kt m", p=128))
            bk_t = wp.tile([128, 2], F32, tag="bk")
            nc.sync.dma_start(bk_t[:], bk_d[:])

            # ---- persistent activation tiles ----
            qT = [actp.tile([128, LQ], F32R, tag=f"qT{p}", name=f"qT{p}")
                  for p in range(2)]
            kT = [actp.tile([128, LQ], F32R, tag=f"kT{p}", name=f"kT{p}")
                  for p in range(2)]
            v_t = actp.tile([128, NLK * VAW], BF16, tag="v")
            aT = [actp.tile([128, LQ], BF16, tag=f"aT{p}", name=f"aT{p}")
                  for p in range(2)]
            id_t = wp.tile([128, 128], BF16, tag="id")

            # warm the ACT exp table during the lead-in DMAs
            warm_t = tmpp.tile([128, 1], F32, tag="warm")
            nc.scalar.activation(warm_t[:], bk_t[:, 0:1], EXP, scale=1.0)

            # ones columns of v_t via one strided DMA
            nc.gpsimd.dma_start(
                v_t[:].rearrange("p (l w) -> p l w", w=VW)[:, :, 0:1],
                vo_d.rearrange("p (l o) -> p l o", o=1))

            wq_t = wp.tile([128, KT_Q * 256], BF16, tag="wq")
            bq_t = wp.tile([128, 2], F32, tag="bq")
            wv_t = wp.tile([128, KT_C * 256], BF16, tag="wv")
            bv_t = wp.tile([128, 256], F32, tag="bv")
            wo_t = wp.tile([128, 2 * D], BF16, tag="wo")
            ct_tiles = {}
            xt_tiles = {}

            def ct_dma(s, eng=None):
                eng = eng or nc.sync
                t = ctp.tile([128, KT_C * 512], BF16, tag="ct")
                tv = t[:].rearrange("p (kt q) -> p kt q", q=512)
                cv = cT_d.rearrange("(kt p) q -> p kt q",
                                    p=128)[:, :, 512 * s:512 * (s + 1)]
                for kk in range(3):
                    eng.dma_start(tv[:, 2 * kk:2 * kk + 2, :],
                                  cv[:, 2 * kk:2 * kk + 2, :])
                ct_tiles[s] = t

            def k_proj(s, p):
                if s not in ct_tiles:
                    ct_dma(s)
                t = ct_tiles[s]
                ps = scr.tile([128, 512], F32, tag="u")
                for kt in range(KT_C):
                    nc.tensor.matmul(
                        ps[:], wk_t[:, 256 * kt + 128 * p:256 * kt + 128 * (p + 1)],
                        t[:, 512 * kt:512 * (kt + 1)],
                        start=(kt == 0), stop=(kt == KT_C - 1))
                nc.vector.tensor_scalar_add(
                    kT[p][:, 512 * s:512 * (s + 1)], ps[:], bk_t[:, p:p + 1])

            def xt_dma(s, eng=None):
                eng = eng or nc.sync
                t = xtp.tile([128, KT_Q * 512], BF16, tag="xt")
                tv = t[:].rearrange("p (kt q) -> p kt q", q=512)
                xv = xT_d.rearrange("(kt p) q -> p kt q",
                                    p=128)[:, :, 512 * s:512 * (s + 1)]
                for kk in range(2):
                    eng.dma_start(tv[:, 4 * kk:4 * kk + 4, :],
                                  xv[:, 4 * kk:4 * kk + 4, :])
                xt_tiles[s] = t

            def q_proj(s, p):
                if s not in xt_tiles:
                    xt_dma(s)
                t = xt_tiles[s]
                ps = scr.tile([128, 512], F32, tag="u")
                for kt in range(KT_Q):
                    nc.tensor.matmul(
                        ps[:], wq_t[:, 256 * kt + 128 * p:256 * kt + 128 * (p + 1)],
                        t[:, 512 * kt:512 * (kt + 1)],
                        start=(kt == 0), stop=(kt == KT_Q - 1))
                nc.vector.tensor_scalar_add(
                    qT[p][:, 512 * s:512 * (s + 1)], ps[:], bq_t[:, p:p + 1])

            def v_chunk(j):
                ps = scr.tile([128, 256], F32, tag="u")
                s, jj = j // 4, j % 4
                for kt in range(KT_C):
                    nc.tensor.matmul(
                        ps[:],
                        ct_tiles[s][:, 512 * kt + 128 * jj:512 * kt + 128 * (jj + 1)],
                        wv_t[:, 256 * kt:256 * (kt + 1)],
                        start=(kt == 0), stop=(kt == KT_C - 1))
                vv = v_t[:, VAW * j:VAW * (j + 1)].rearrange(
                    "p (h w) -> p h w", w=VW)[:, :, 1:VW]
                nc.vector.tensor_add(
                    vv, ps[:].rearrange("p (h w) -> p h w", w=HD),
                    bv_t[:].rearrange("p (h w) -> p h w", w=HD))

            # ---- per-(half, head) attention state ----
            avt = [avp.tile([128, 512], F32, tag=f"avt{i}", name=f"avt{i}")
                   for i in range(2)]

            def avt_run(ctx, qb):
                """One transposed-AV accumulation run for a retiring head."""
                exs, h_ = ctx["exs"], ctx["h"]
                ps = avt[qb // 4]
                off = VW * (qb % 4)
                for j in range(NLK):
                    nc.tensor.matmul(
                        ps[:, off:off + VW],
                        exs[j][:, 128 * qb:128 * (qb + 1)],
                        v_t[:, VAW * j + VW * h_:VAW * j + VW * (h_ + 1)],
                        start=(j == 0), stop=(j == NLK - 1))

            def avt_recip(ctx, bank):
                """1/d for the 4 qb blocks of one avt bank."""
                rd = rdp.tile([128, 4], F32, tag=f"rd{bank}")
                dg = avt[bank][:, 0:4 * VW:VW]
                nc.vector.reciprocal_approx_fast(rd[:], dg)
                ctx[f"rd{bank}"] = rd

            def avt_norm(ctx, bank):
                """normalize a whole avt bank -> attn_sb bf16 [128q, 4, 64]."""
                rd = ctx[f"rd{bank}"]
                at = attnp.tile([128, 4 * HD], BF16, tag="at")
                src = avt[bank][:, 0:4 * VW].rearrange(
                    "p (b w) -> p b w", w=VW)[:, :, 1:VW]
                rdb = rd[:].rearrange("p (b o) -> p b o", o=1).broadcast_to(
                    [128, 4, HD])
                nc.vector.tensor_mul(
                    at[:].rearrange("p (b w) -> p b w", w=HD), src, rdb)
                ctx[f"at{bank}"] = at

            def avt_tr(ctx, qb):
                """PE-transpose attn qb into the bank's tp psum tile."""
                half_, h_ = ctx["half"], ctx["h"]
                m_ = h_ % 2
                bank = qb // 4
                at = ctx[f"at{bank}"]
                if qb % 4 == 0:
                    tpn = scr.tile([128, 4 * 128], BF16, tag="u")
                    ctx[f"tp{bank}"] = tpn
                tp = ctx[f"tp{bank}"]
                nc.tensor.matmul(
                    tp[64 * m_:64 * (m_ + 1), 128 * (qb % 4):128 * (qb % 4 + 1)],
                    at[:, HD * (qb % 4):HD * (qb % 4 + 1)], id_t[:],
                    is_transpose=True)

            def avt_cp(ctx, bank):
                """copy a bank of transposed attn into aT."""
                half_, h_ = ctx["half"], ctx["h"]
                p_, m_ = h_ // 2, h_ % 2
                tp = ctx.pop(f"tp{bank}")
                nc.vector.tensor_copy(
                    aT[p_][64 * m_:64 * (m_ + 1),
                           HALF * half_ + 512 * bank:HALF * half_ + 512 * (bank + 1)],
                    tp[64 * m_:64 * (m_ + 1), :])

            def out_unit(mo, s, p_, on_act=False):
                """out-proj partial unit for pair p_: one MM + copy + DMA."""
                ps = scr.tile([128, 512], F32, tag="u")
                nc.tensor.matmul(
                    ps[:], wo_t[:, D * p_ + 128 * mo:D * p_ + 128 * (mo + 1)],
                    aT[p_][:, 512 * s:512 * (s + 1)],
                    start=True, stop=True)
                ot = outp.tile([128, 512], BF16, tag="out")
                if on_act:
                    nc.scalar.copy(ot[:], ps[:])
                else:
                    nc.vector.tensor_copy(ot[:], ps[:])
                dst = out_d if p_ == 0 else out2_d
                nc.gpsimd.dma_start(
                    dst[128 * mo:128 * (mo + 1), 512 * s:512 * (s + 1)], ot[:])

            # ---- lead-in: DMAs spread across engine queues ----
            nc.scalar.dma_start(wq_t[:].rearrange("p (kt m) -> p kt m", m=256),
                                wq_d.rearrange("(kt p) m -> p kt m", p=128))
            nc.sync.dma_start(bq_t[:], bq_d[:])
            xt_dma(0)                       # SP
            ct_dma(0, nc.gpsimd)            # Pool (behind wk)
            xt_dma(1, nc.scalar)            # ACT (behind wq)
            ct_dma(1, nc.gpsimd)
            nc.gpsimd.dma_start(wv_t[:].rearrange("p (kt m) -> p kt m", m=256),
                                wv_d.rearrange("(kt p) m -> p kt m", p=128))
            nc.gpsimd.dma_start(bv_t[:], bv_d[:])
            nc.gpsimd.dma_start(id_t[:], id_d[:])
            nc.gpsimd.dma_start(wo_t[:].rearrange("p (p2 m) -> p p2 m", m=1024),
                                wo_d.rearrange("(p2 p) m -> p p2 m", p=128))
            k_proj(0, 0)
            q_proj(0, 0)
            q_proj(1, 0)

            # ---- drip worklists per window (list of (pos, 0-arg callable)) --
            def W(fn, *a):
                return lambda: fn(*a)

            def retire_steps(ctx, t0, dt):
                """AV_T + recip + norm + transpose steps for a finished head,
                spread from emission position t0 with spacing dt."""
                st = []
                t = t0
                for b in range(2):
                    for qq in range(4):
                        st.append((t, W(avt_run, ctx, 4 * b + qq)))
                        t += dt
                    st.append((t, W(avt_recip, ctx, b)))
                    t += dt / 4
                for bank in range(2):
                    st.append((t, W(avt_norm, ctx, bank)))
                    t += dt / 3
                    for qq in range(4):
                        st.append((t, W(avt_tr, ctx, 4 * bank + qq)))
                        t += dt / 3
                    st.append((t, W(avt_cp, ctx, bank)))
                    t += dt / 3
                return st

            # DVE-offloaded exp tiles per window
            DVE_JS = set()

            def emit_exp(st, j, on_dve):
                ex = expp.tile([128, HALF], BF16, tag="expS")
                if on_dve:
                    stg = tmpp.tile([128, HALF], F32, tag="stg")
                    nc.scalar.copy(stg[:], st[:])
                    tmp = tmpp.tile([128, HALF], F32, tag="tmp")
                    nc.vector._custom_dve(EXP_A, out=tmp[:], in0=stg[:],
                                          s0=EC1, s1=EC2, imm2=EC3)
                    nc.vector._custom_dve(EXP_B, out=ex[:], in0=tmp[:])
                else:
                    nc.scalar.activation(ex[:], st[:], EXP, scale=32.0)
                return ex

            # static drips (beyond the retire pipeline), per window
            wl = [[] for _ in range(9)]
            wl[0] = [
                (1, W(ct_dma, 2)), (2, W(k_proj, 1, 0)), (3, W(v_chunk, 0)),
                (4, W(v_chunk, 1)), (4.5, W(ct_dma, 3)), (5, W(k_proj, 2, 0)),
                (6, W(v_chunk, 2)), (7, W(v_chunk, 3)), (8, W(v_chunk, 4)),
                (9, W(k_proj, 3, 0)), (10, W(v_chunk, 5)), (11, W(v_chunk, 6)),
                (12, W(v_chunk, 7)), (13, W(v_chunk, 8)), (14, W(v_chunk, 9)),
            ]
            wl[1] = [
                (0, W(v_chunk, 10)), (1, W(v_chunk, 11)), (2, W(v_chunk, 12)),
                (3, W(v_chunk, 13)), (4, W(v_chunk, 14)), (5, W(v_chunk, 15)),
                (5.5, W(k_proj, 0, 1)), (8, W(q_proj, 0, 1)),
                (11, W(q_proj, 1, 1)),
            ]
            wl[2] = [(1, W(k_proj, 1, 1)),
                     (6, W(k_proj, 2, 1)), (10, W(k_proj, 3, 1))]
            wl[3] = ([(2, W(q_proj, 2, 0)), (5, W(q_proj, 3, 0))]
                     + [(8 + m, W(out_unit, m, m % 2, 0)) for m in range(8)])
            wl[4] = ([(2, W(q_proj, 2, 1)), (5, W(q_proj, 3, 1))]
                     + [(8 + m, W(out_unit, m, 1 - m % 2, 0)) for m in range(8)])
            wl[5] = ([(4 + m, W(out_unit, m, m % 2, 1)) for m in range(8)]
                     + [(12.5 + m / 4, W(out_unit, m, 1 - m % 2, 1))
                        for m in range(4)])
            wl[6] = ([(4 + m, W(out_unit, m, 1 - m % 2, 1)) for m in range(4, 8)]
                     + [(9 + m, W(out_unit, m, 2 + m % 2, 0)) for m in range(4)])
            wl[7] = ([(2 + m, W(out_unit, m, 2 + m % 2, 0)) for m in range(4, 8)]
                     + [(8 + m / 2, W(out_unit, m, 3 - m % 2, 0))
                        for m in range(8)])
            wl[8] = []

            # ---- main windows ----
            ctx_prev = None
            for w in range(8):
                half, h = w // 4, w % 4
                p, m = h // 2, h % 2
                r0 = 64 * m
                work = list(wl[w])
                if ctx_prev is not None:
                    # w1: v_chunks land j0-5, retire after; else spread early
                    work += retire_steps(ctx_prev, 6.0 if w == 1 else 0.5, 0.6)
                work.sort(key=lambda t: t[0])
                wi = 0
                exs = []
                for j in range(NLK):
                    while wi < len(work) and work[wi][0] <= j:
                        work[wi][1]()
                        wi += 1
                    st = stp.tile([128, HALF], F32, tag="st")
                    for n in range(2):
                        nc.tensor.matmul(
                            st[:, 512 * n:512 * (n + 1)],
                            kT[p][r0:r0 + 64, 128 * j:128 * (j + 1)],
                            qT[p][r0:r0 + 64,
                                  HALF * half + 512 * n:HALF * half + 512 * (n + 1)],
                            start=True, stop=True)
                    exs.append(emit_exp(st, j, j in DVE_JS))
                while wi < len(work):
                    work[wi][1]()
                    wi += 1
                ctx_prev = {"exs": exs, "half": half, "h": h}

            # ---- tail: retire h3-half1, rest of p0 partials, p1 partials ----
            tail = wl[8] + retire_steps(ctx_prev, 0.0, 0.4)
            tail.sort(key=lambda t: t[0])
            for _, cb in tail:
                cb()
            for mo in range(8):
                out_unit(mo, 2, 1, on_act=True)
                out_unit(mo, 3, 1, on_act=(mo % 2 == 0))

            if DEBUG:
                dbg_a = nc.dram_tensor("dbg_aT0", (128, LQ), F32,
                                       kind="ExternalOutput").ap()
                dbg_q = nc.dram_tensor("dbg_qT0", (128, LQ), F32,
                                       kind="ExternalOutput").ap()
                dbg_k = nc.dram_tensor("dbg_kT0", (128, LQ), F32,
                                       kind="ExternalOutput").ap()
                dbg_a1 = nc.dram_tensor("dbg_aT1", (128, LQ), F32,
                                        kind="ExternalOutput").ap()
                for nm, dst, src in (("a", dbg_a, aT[0]), ("q", dbg_q, qT[0]),
                                     ("k", dbg_k, kT[0]), ("a1", dbg_a1, aT[1])):
                    for c in range(4):
                        t = outp.tile([128, 512], F32, tag="dbg")
                        nc.vector.tensor_copy(t[:], src[:, 512 * c:512 * (c + 1)])
                        nc.gpsimd.dma_start(dst[:, 512 * c:512 * (c + 1)], t[:])

    nc.compile()
    return nc


_NC_CACHE = []


def _get_nc():
    if not _NC_CACHE:
        _NC_CACHE.append(_build())
    return _NC_CACHE[0]


OUT_NAME = "outT"


def prep_maps(inputs):
    """Host-side prep: per-core input tensor maps."""
    import ml_dtypes
    bf16 = ml_dtypes.bfloat16
    x = np.asarray(inputs["x"], np.float32)
    context = np.asarray(inputs["context"], np.float32)
    w_q = np.asarray(inputs["w_q"], np.float32)
    b_q = np.asarray(inputs["b_q"], np.float32)
    w_k = np.asarray(inputs["w_k"], np.float32)
    b_k = np.asarray(inputs["b_k"], np.float32)
    w_v = np.asarray(inputs["w_v"], np.float32)
    b_v = np.asarray(inputs["b_v"], np.float32)
    w_o = np.asarray(inputs["w_o"], np.float32)

    xTb = [np.ascontiguousarray(x[b].T).astype(bf16) for b in range(B)]
    cTb = [np.ascontiguousarray(context[b].T).astype(bf16) for b in range(B)]
    ident = np.eye(128, dtype=np.float32).astype(bf16)
    vones = np.ones((128, NLK * GH), np.float32).astype(bf16)
    maps = []
    for c in range(8):
        b, g = c // 4, c % 4
        hs = slice(256 * g, 256 * (g + 1))
        maps.append({
            "xT": xTb[b],
            "ctxT": cTb[b],
            # scale by 2^-8 (exact in fp): exp scale 32 * score scale 1/8
            "wq": (np.ascontiguousarray(w_q[:, hs]) / 256.0).astype(bf16),
            "wk": np.ascontiguousarray(w_k[:, hs]).astype(bf16),
            "wv": np.ascontiguousarray(w_v[:, hs]).astype(bf16),
            "wo": np.ascontiguousarray(w_o[hs, :]).astype(bf16),
            "bq": np.ascontiguousarray((b_q[hs] / 256.0).reshape(2, 128).T),
            "bk": np.ascontiguousarray(b_k[hs].reshape(2, 128).T),
            "bv": np.broadcast_to(b_v[None, hs], (128, 256)).copy(),
            "ident": ident,
            "vones": vones,
        })
    return maps


def kernel_run(inputs, trace=False, **kw):
    """Run on HW; returns (full_output, BassKernelResults)."""
    b_o = np.asarray(inputs["b_o"], np.float32)
    maps = prep_maps(inputs)
    nc = _get_nc()
    res = bass_utils.run_bass_kernel_spmd(nc, maps, core_ids=list(range(8)),
                                          trace=trace, **kw)
    out = np.empty((B, LQ, D), np.float32)
    for b in range(B):
        acc = res.results[4 * b]["outT"].astype(np.float32)
        acc += res.results[4 * b]["outT2"]
        for g in range(1, 4):
            acc = acc + res.results[4 * b + g]["outT"]
            acc = acc + res.results[4 * b + g]["outT2"]
        out[b] = acc.T + b_o[None, :]
    return out, res


def kernel(**inputs) -> np.ndarray:
    out, _ = kernel_run(inputs)
    return out


# revision 30
# speedup vs baseline: 1.1070x; 1.0135x over previous
"""Cross-attention Trainium2 kernel (nn_CrossAttention, B=2, L=2048, D=1024,
Dctx=768, 16 heads x 64).

Sharding: 8 cores = 2 (batch) x 4 (head-groups of 4 heads). Each core computes
its batch's Q/K/V projections for its 4 heads, attention, and a partial output
projection; the host sums the head-group partials and adds b_o.

v2 design (CoreSim cost model):
- AV computed TRANSPOSED: per (half, head, q-block of 128), accumulate
  out_T[q128, 65] = sum_j ex_j[qb-chunk].T @ [ones|v_j] over the 16 key
  blocks, with ex as the (free-to-reload) stationary operand. Cost-model
  charge is out-free-size (65) per matmul, halving AV cost vs the
  untransposed form. Column 0 carries the softmax denominator.
- Per-qb normalize (reciprocal_approx_fast on a strided d-gather + one
  tensor_scalar_mul), then a PE transpose (identity matmul) back to
  [64, q] and a DVE copy into aT for the output projection.
- exp runs on ACT (scale=32; wq/bq pre-scaled by 1/256 on host so scores
  arrive as u/32) with a few tiles offloaded to 2-inst custom-DVE
  polynomial exp (p3(u/32)^32, rel err ~5e-4) to balance engines.
- ex tiles are bf16 (stationary operand; halves SBUF so a full head-half
  of 16 tiles stays live for the lagged transposed-AV pass).
- Tail split: the final pair's (s2,s3) out-projection contraction is
  emitted as two bf16 partials (outT + outT2) summed on host, so the p0
  half runs a window early instead of serializing in the tail.
"""
import numpy as np

import concourse.bass as bass
import concourse.tile as tile
from concourse import bacc, mybir, bass_utils

# ---- custom DVE exp ops (runtime registration, documented extension path) ---
import concourse.dve_ops as dve_ops
from concourse.dve_ops import DveOp, OPS, CUSTOM_DVE_SPECS, _SUB_OPCODE_FOR_NAME
from concourse.dve_spec import Spec, Src0, C0, C1, C2, One, lower, sq
from concourse.dve_uop import DveOpSpec

_t = Src0
_p3 = One + _t * (C0 + _t * (C1 + _t * C2))


def _ref_exp_a(in0, in1, c0, c1, c2):
    t = in0.astype(np.float32)
    p = 1.0 + t * (np.float32(c0) + t * (np.float32(c1) + t * np.float32(c2)))
    return (p * p).astype(np.float32)


def _ref_exp_b(in0, in1, c0, c1, c2):
    y = in0.astype(np.float32)
    y = y * y
    y = y * y
    y = y * y
    return (y * y).astype(np.float32)


def _register(name, spec):
    if name in _SUB_OPCODE_FOR_NAME:
        return next(o for o in OPS if o.name == name)
    row = dve_ops._CUSTOM_DVE_ROW_BASE + len(OPS)
    assert row < 0x20
    _SUB_OPCODE_FOR_NAME[name] = row
    shas = {}
    for ver in ("v3", "v4"):
        s = DveOpSpec(name=name, opcode=row, uops=lower(spec, ver=ver),
                      rd1_en=False)
        shas[ver] = s.sha(ver)
    op = DveOp(name, spec, subdim=False, uops_sha=shas)
    OPS.append(op)
    CUSTOM_DVE_SPECS[name] = spec
    return op


EXP_A = _register("ANT_EXP_P3SQ1", Spec(body=sq(_p3), reference=_ref_exp_a))
EXP_B = _register("ANT_SQ4", Spec(body=sq(sq(sq(sq(Src0)))), reference=_ref_exp_b))

# minimax-ish p3 for e^t on [-0.27, 0.27] (c0 normalized to 1; the global
# p(0)^32 factor cancels in softmax): c1, c2, c3
EC1, EC2, EC3 = 1.00005423, 0.50272472, 0.16640462

F32R = mybir.dt.float32r
F32 = mybir.dt.float32
BF16 = mybir.dt.bfloat16
EXP = mybir.ActivationFunctionType.Exp

# Problem shape (hardcoded per harness contract)
B, LQ, D = 2, 2048, 1024
DCTX = 768
NH, HD = 16, 64
SCALE = 1.0 / 8.0

GH = 4                # heads per core
VW = HD + 1           # 65: [ones | v] lane per (j, h)
VAW = GH * VW         # 260
KT_Q = D // 128       # 8
KT_C = DCTX // 128    # 6
NLK = LQ // 128       # 16 key blocks
NQB = 8               # q-blocks of 128 per half
HALF = 1024


DEBUG = False


def _build():
    nc = bacc.Bacc("TRN2", target_bir_lowering=False, debug=False,
                   enable_asserts=False, num_devices=8)

    xT_d = nc.dram_tensor("xT", (D, LQ), BF16, kind="ExternalInput").ap()
    cT_d = nc.dram_tensor("ctxT", (DCTX, LQ), BF16, kind="ExternalInput").ap()
    wq_d = nc.dram_tensor("wq", (D, 256), BF16, kind="ExternalInput").ap()
    wk_d = nc.dram_tensor("wk", (DCTX, 256), BF16, kind="ExternalInput").ap()
    wv_d = nc.dram_tensor("wv", (DCTX, 256), BF16, kind="ExternalInput").ap()
    wo_d = nc.dram_tensor("wo", (256, D), BF16, kind="ExternalInput").ap()
    bq_d = nc.dram_tensor("bq", (128, 2), F32, kind="ExternalInput").ap()
    bk_d = nc.dram_tensor("bk", (128, 2), F32, kind="ExternalInput").ap()
    bv_d = nc.dram_tensor("bv", (128, 256), F32, kind="ExternalInput").ap()
    id_d = nc.dram_tensor("ident", (128, 128), BF16, kind="ExternalInput").ap()
    vo_d = nc.dram_tensor("vones", (128, NLK * GH), BF16,
                          kind="ExternalInput").ap()
    out_d = nc.dram_tensor("outT", (D, LQ), BF16, kind="ExternalOutput").ap()
    out2_d = nc.dram_tensor("outT2", (D, LQ), BF16, kind="ExternalOutput").ap()

    with tile.TileContext(nc) as tc:
        with tc.tile_pool(name="w", bufs=1) as wp, \
             tc.tile_pool(name="xt", bufs=2) as xtp, \
             tc.tile_pool(name="ct", bufs=4) as ctp, \
             tc.tile_pool(name="act", bufs=1) as actp, \
             tc.tile_pool(name="expp", bufs=34) as expp, \
             tc.tile_pool(name="tmpp", bufs=2) as tmpp, \
             tc.tile_pool(name="attn", bufs=6) as attnp, \
             tc.tile_pool(name="rdp", bufs=4) as rdp, \
             tc.tile_pool(name="outp", bufs=6) as outp, \
             tc.tile_pool(name="stp", bufs=2, space="PSUM") as stp, \
             tc.tile_pool(name="avp", bufs=1, space="PSUM") as avp, \
             tc.tile_pool(name="scr", bufs=2, space="PSUM") as scr:

            # ---- earliest deps first: K path ----
            wk_t = wp.tile([128, KT_C * 256], BF16, tag="wk")
            nc.gpsimd.dma_start(wk_t[:].rearrange("p (kt m) -> p kt m", m=256),
                                wk_d.rearrange("(kt p) m -> p kt m", p=128))
            bk_t = wp.tile([128, 2], F32, tag="bk")
            nc.scalar.dma_start(bk_t[:], bk_d[:])

            # ---- persistent activation tiles ----
            qT = [actp.tile([128, LQ], F32R, tag=f"qT{p}", name=f"qT{p}")
                  for p in range(2)]
            kT = [actp.tile([128, LQ], F32R, tag=f"kT{p}", name=f"kT{p}")
                  for p in range(2)]
            v_t = actp.tile([128, NLK * VAW], BF16, tag="v")
            aT = [actp.tile([128, LQ], BF16, tag=f"aT{p}", name=f"aT{p}")
                  for p in range(2)]
            id_t = wp.tile([128, 128], BF16, tag="id")

            # warm the ACT exp table during the lead-in DMAs
            warm_t = tmpp.tile([128, 1], F32, tag="warm")
            nc.scalar.activation(warm_t[:], bk_t[:, 0:1], EXP, scale=1.0)

            # ones columns of v_t via one strided DMA
            nc.scalar.dma_start(
                v_t[:].rearrange("p (l w) -> p l w", w=VW)[:, :, 0:1],
                vo_d.rearrange("p (l o) -> p l o", o=1))

            wq_t = wp.tile([128, KT_Q * 256], BF16, tag="wq")
            bq_t = wp.tile([128, 2], F32, tag="bq")
            wv_t = wp.tile([128, KT_C * 256], BF16, tag="wv")
            bv_t = wp.tile([128, 256], F32, tag="bv")
            wo_t = wp.tile([128, 2 * D], BF16, tag="wo")
            ct_tiles = {}
            xt_tiles = {}

            def ct_dma(s, eng=None):
                eng = eng or nc.sync
                t = ctp.tile([128, KT_C * 512], BF16, tag="ct")
                tv = t[:].rearrange("p (kt q) -> p kt q", q=512)
                cv = cT_d.rearrange("(kt p) q -> p kt q",
                                    p=128)[:, :, 512 * s:512 * (s + 1)]
                for kk in range(3):
                    eng.dma_start(tv[:, 2 * kk:2 * kk + 2, :],
                                  cv[:, 2 * kk:2 * kk + 2, :])
                ct_tiles[s] = t

            def k_proj(s, p):
                if s not in ct_tiles:
                    ct_dma(s)
                t = ct_tiles[s]
                ps = scr.tile([128, 512], F32, tag="u")
                for kt in range(KT_C):
                    nc.tensor.matmul(
                        ps[:], wk_t[:, 256 * kt + 128 * p:256 * kt + 128 * (p + 1)],
                        t[:, 512 * kt:512 * (kt + 1)],
                        start=(kt == 0), stop=(kt == KT_C - 1))
                nc.vector.tensor_scalar_add(
                    kT[p][:, 512 * s:512 * (s + 1)], ps[:], bk_t[:, p:p + 1])

            def xt_dma(s, eng=None):
                eng = eng or nc.sync
                t = xtp.tile([128, KT_Q * 512], BF16, tag="xt")
                tv = t[:].rearrange("p (kt q) -> p kt q", q=512)
                xv = xT_d.rearrange("(kt p) q -> p kt q",
                                    p=128)[:, :, 512 * s:512 * (s + 1)]
                for kk in range(2):
                    eng.dma_start(tv[:, 4 * kk:4 * kk + 4, :],
                                  xv[:, 4 * kk:4 * kk + 4, :])
                xt_tiles[s] = t

            def q_proj(s, p):
                if s not in xt_tiles:
                    xt_dma(s)
                t = xt_tiles[s]
                ps = scr.tile([128, 512], F32, tag="u")
                for kt in range(KT_Q):
                    nc.tensor.matmul(
                        ps[:], wq_t[:, 256 * kt + 128 * p:256 * kt + 128 * (p + 1)],
                        t[:, 512 * kt:512 * (kt + 1)],
                        start=(kt == 0), stop=(kt == KT_Q - 1))
                nc.vector.tensor_scalar_add(
                    qT[p][:, 512 * s:512 * (s + 1)], ps[:], bq_t[:, p:p + 1])

            def v_chunk(j):
                ps = scr.tile([128, 256], F32, tag="u")
                s, jj = j // 4, j % 4
                for kt in range(KT_C):
                    nc.tensor.matmul(
                        ps[:],
                        ct_tiles[s][:, 512 * kt + 128 * jj:512 * kt + 128 * (jj + 1)],
                        wv_t[:, 256 * kt:256 * (kt + 1)],
                        start=(kt == 0), stop=(kt == KT_C - 1))
                vv = v_t[:, VAW * j:VAW * (j + 1)].rearrange(
                    "p (h w) -> p h w", w=VW)[:, :, 1:VW]
                nc.vector.tensor_add(
                    vv, ps[:].rearrange("p (h w) -> p h w", w=HD),
                    bv_t[:].rearrange("p (h w) -> p h w", w=HD))

            # ---- per-(half, head) attention state ----
            avt = [avp.tile([128, 512], F32, tag=f"avt{i}", name=f"avt{i}")
                   for i in range(2)]

            def avt_run(ctx, qb):
                """One transposed-AV accumulation run for a retiring head."""
                exs, h_ = ctx["exs"], ctx["h"]
                ps = avt[qb // 4]
                off = VW * (qb % 4)
                for j in range(NLK):
                    nc.tensor.matmul(
                        ps[:, off:off + VW],
                        exs[j][:, 128 * qb:128 * (qb + 1)],
                        v_t[:, VAW * j + VW * h_:VAW * j + VW * (h_ + 1)],
                        start=(j == 0), stop=(j == NLK - 1))

            def avt_recip(ctx, bank):
                """1/d for the 4 qb blocks of one avt bank."""
                rd = rdp.tile([128, 4], F32, tag=f"rd{bank}")
                dg = avt[bank][:, 0:4 * VW:VW]
                nc.vector.reciprocal_approx_fast(rd[:], dg)
                ctx[f"rd{bank}"] = rd

            def avt_norm(ctx, bank):
                """normalize a whole avt bank -> attn_sb bf16 [128q, 4, 64]."""
                rd = ctx[f"rd{bank}"]
                at = attnp.tile([128, 4 * HD], BF16, tag="at")
                src = avt[bank][:, 0:4 * VW].rearrange(
                    "p (b w) -> p b w", w=VW)[:, :, 1:VW]
                rdb = rd[:].rearrange("p (b o) -> p b o", o=1).broadcast_to(
                    [128, 4, HD])
                nc.vector.tensor_mul(
                    at[:].rearrange("p (b w) -> p b w", w=HD), src, rdb)
                ctx[f"at{bank}"] = at

            def avt_tr(ctx, qb):
                """PE-transpose attn qb into the bank's tp psum tile."""
                half_, h_ = ctx["half"], ctx["h"]
                m_ = h_ % 2
                bank = qb // 4
                at = ctx[f"at{bank}"]
                if qb % 4 == 0:
                    tpn = scr.tile([128, 4 * 128], BF16, tag="u")
                    ctx[f"tp{bank}"] = tpn
                tp = ctx[f"tp{bank}"]
                nc.tensor.matmul(
                    tp[64 * m_:64 * (m_ + 1), 128 * (qb % 4):128 * (qb % 4 + 1)],
                    at[:, HD * (qb % 4):HD * (qb % 4 + 1)], id_t[:],
                    is_transpose=True)

            def avt_cp(ctx, bank):
                """copy a bank of transposed attn into aT."""
                half_, h_ = ctx["half"], ctx["h"]
                p_, m_ = h_ // 2, h_ % 2
                tp = ctx.pop(f"tp{bank}")
                nc.vector.tensor_copy(
                    aT[p_][64 * m_:64 * (m_ + 1),
                           HALF * half_ + 512 * bank:HALF * half_ + 512 * (bank + 1)],
                    tp[64 * m_:64 * (m_ + 1), :])

            def out_unit(mo, s, p_, on_act=False):
                """out-proj partial unit for pair p_: one MM + copy + DMA."""
                ps = scr.tile([128, 512], F32, tag="u")
                nc.tensor.matmul(
                    ps[:], wo_t[:, D * p_ + 128 * mo:D * p_ + 128 * (mo + 1)],
                    aT[p_][:, 512 * s:512 * (s + 1)],
                    start=True, stop=True)
                ot = outp.tile([128, 512], BF16, tag="out")
                if on_act:
                    nc.scalar.copy(ot[:], ps[:])
                else:
                    nc.vector.tensor_copy(ot[:], ps[:])
                dst = out_d if p_ == 0 else out2_d
                nc.gpsimd.dma_start(
                    dst[128 * mo:128 * (mo + 1), 512 * s:512 * (s + 1)], ot[:])

            # ---- lead-in: DMAs spread across engine queues ----
            nc.scalar.dma_start(wq_t[:].rearrange("p (kt m) -> p kt m", m=256),
                                wq_d.rearrange("(kt p) m -> p kt m", p=128))
            xt_dma(0)                       # SP
            ct_dma(0, nc.gpsimd)            # Pool (behind wk)
            nc.scalar.dma_start(bq_t[:], bq_d[:])
            xt_dma(1)                       # SP
            ct_dma(1, nc.gpsimd)
            nc.scalar.dma_start(id_t[:], id_d[:])
            nc.gpsimd.dma_start(wv_t[:].rearrange("p (kt m) -> p kt m", m=256),
                                wv_d.rearrange("(kt p) m -> p kt m", p=128))
            nc.scalar.dma_start(bv_t[:], bv_d[:])
            nc.gpsimd.dma_start(wo_t[:].rearrange("p (p2 m) -> p p2 m", m=1024),
                                wo_d.rearrange("(p2 p) m -> p p2 m", p=128))
            k_proj(0, 0)
            q_proj(0, 0)
            q_proj(1, 0)

            # ---- drip worklists per window (list of (pos, 0-arg callable)) --
            def W(fn, *a):
                return lambda: fn(*a)

            def retire_steps(ctx, t0, dt):
                """AV_T + recip + norm + transpose steps for a finished head,
                spread from emission position t0 with spacing dt."""
                st = []
                t = t0
                for b in range(2):
                    for qq in range(4):
                        st.append((t, W(avt_run, ctx, 4 * b + qq)))
                        t += dt
                    st.append((t, W(avt_recip, ctx, b)))
                    t += dt / 4
                for bank in range(2):
                    st.append((t, W(avt_norm, ctx, bank)))
                    t += dt / 3
                    for qq in range(4):
                        st.append((t, W(avt_tr, ctx, 4 * bank + qq)))
                        t += dt / 3
                    st.append((t, W(avt_cp, ctx, bank)))
                    t += dt / 3
                return st

            # DVE-offloaded exp tiles per window
            DVE_JS = set()

            def emit_exp(st, j, on_dve):
                ex = expp.tile([128, HALF], BF16, tag="expS")
                if on_dve:
                    stg = tmpp.tile([128, HALF], F32, tag="stg")
                    nc.scalar.copy(stg[:], st[:])
                    tmp = tmpp.tile([128, HALF], F32, tag="tmp")
                    nc.vector._custom_dve(EXP_A, out=tmp[:], in0=stg[:],
                                          s0=EC1, s1=EC2, imm2=EC3)
                    nc.vector._custom_dve(EXP_B, out=ex[:], in0=tmp[:])
                else:
                    nc.scalar.activation(ex[:], st[:], EXP, scale=32.0)
                return ex

            # static drips (beyond the retire pipeline), per window
            wl = [[] for _ in range(9)]
            wl[0] = [
                (1, W(ct_dma, 2)), (2, W(k_proj, 1, 0)), (3, W(v_chunk, 0)),
                (4, W(v_chunk, 1)), (4.5, W(ct_dma, 3)), (5, W(k_proj, 2, 0)),
                (6, W(v_chunk, 2)), (7, W(v_chunk, 3)), (8, W(v_chunk, 4)),
                (9, W(k_proj, 3, 0)), (10, W(v_chunk, 5)), (11, W(v_chunk, 6)),
                (12, W(v_chunk, 7)), (13, W(v_chunk, 8)), (14, W(v_chunk, 9)),
            ]
            wl[1] = [
                (0, W(v_chunk, 10)), (1, W(v_chunk, 11)), (2, W(v_chunk, 12)),
                (3, W(v_chunk, 13)), (4, W(v_chunk, 14)), (5, W(v_chunk, 15)),
                (5.5, W(k_proj, 0, 1)), (8, W(q_proj, 0, 1)),
                (11, W(q_proj, 1, 1)),
            ]
            wl[2] = [(1, W(k_proj, 1, 1)),
                     (6, W(k_proj, 2, 1)), (10, W(k_proj, 3, 1))]
            wl[3] = ([(2, W(q_proj, 2, 0)), (5, W(q_proj, 3, 0))]
                     + [(8 + m, W(out_unit, m, m % 2, 0)) for m in range(8)])
            wl[4] = ([(2, W(q_proj, 2, 1)), (5, W(q_proj, 3, 1))]
                     + [(8 + m, W(out_unit, m, 1 - m % 2, 0)) for m in range(8)])
            wl[5] = ([(4 + m, W(out_unit, m, m % 2, 1)) for m in range(8)]
                     + [(12.5 + m / 4, W(out_unit, m, 1 - m % 2, 1))
                        for m in range(4)])
            wl[6] = ([(4 + m, W(out_unit, m, 1 - m % 2, 1)) for m in range(4, 8)]
                     + [(9 + m, W(out_unit, m, 2 + m % 2, 0)) for m in range(4)])
            wl[7] = ([(2 + m, W(out_unit, m, 2 + m % 2, 0)) for m in range(4, 8)]
                     + [(8 + m / 2, W(out_unit, m, 3 - m % 2, 0))
                        for m in range(8)])
            wl[8] = []

            # ---- main windows ----
            ctx_prev = None
            for w in range(8):
                half, h = w // 4, w % 4
                p, m = h // 2, h % 2
                r0 = 64 * m
                work = list(wl[w])
                if ctx_prev is not None:
                    # w1: v_chunks land j0-5, retire after; else spread early
                    work += retire_steps(ctx_prev, 6.0 if w == 1 else 0.5, 0.6)
                work.sort(key=lambda t: t[0])
                wi = 0
                exs = []
                for j in range(NLK):
                    while wi < len(work) and work[wi][0] <= j:
                        work[wi][1]()
                        wi += 1
                    st = stp.tile([128, HALF], F32, tag="st")
                    for n in range(2):
                        nc.tensor.matmul(
                            st[:, 512 * n:512 * (n + 1)],
                            kT[p][r0:r0 + 64, 128 * j:128 * (j + 1)],
                            qT[p][r0:r0 + 64,
                                  HALF * half + 512 * n:HALF * half + 512 * (n + 1)],
                            start=True, stop=True)
                    exs.append(emit_exp(st, j, j in DVE_JS))
                while wi < len(work):
                    work[wi][1]()
                    wi += 1
                ctx_prev = {"exs": exs, "half": half, "h": h}

            # ---- tail: retire h3-half1, rest of p0 partials, p1 partials ----
            tail = wl[8] + retire_steps(ctx_prev, 0.0, 0.4)
            tail.sort(key=lambda t: t[0])
            for _, cb in tail:
                cb()
            for mo in range(8):
                out_unit(mo, 2, 1, on_act=True)
                out_unit(mo, 3, 1, on_act=(mo % 2 == 0))

            if DEBUG:
                dbg_a = nc.dram_tensor("dbg_aT0", (128, LQ), F32,
                                       kind="ExternalOutput").ap()
                dbg_q = nc.dram_tensor("dbg_qT0", (128, LQ), F32,
                                       kind="ExternalOutput").ap()
                dbg_k = nc.dram_tensor("dbg_kT0", (128, LQ), F32,
                                       kind="ExternalOutput").ap()
                dbg_a1 = nc.dram_tensor("dbg_aT1", (128, LQ), F32,
                                        kind="ExternalOutput").ap()
                for nm, dst, src in (("a", dbg_a, aT[0]), ("q", dbg_q, qT[0]),
                                     ("k", dbg_k, kT[0]), ("a1", dbg_a1, aT[1])):
                    for c in range(4):
                        t = outp.tile([128, 512], F32, tag="dbg")
                        nc.vector.tensor_copy(t[:], src[:, 512 * c:512 * (c + 1)])
                        nc.gpsimd.dma_start(dst[:, 512 * c:512 * (c + 1)], t[:])

    nc.compile()
    return nc


_NC_CACHE = []


def _get_nc():
    if not _NC_CACHE:
        _NC_CACHE.append(_build())
    return _NC_CACHE[0]


OUT_NAME = "outT"


def prep_maps(inputs):
    """Host-side prep: per-core input tensor maps."""
    import ml_dtypes
    bf16 = ml_dtypes.bfloat16
    x = np.asarray(inputs["x"], np.float32)
    context = np.asarray(inputs["context"], np.float32)
    w_q = np.asarray(inputs["w_q"], np.float32)
    b_q = np.asarray(inputs["b_q"], np.float32)
    w_k = np.asarray(inputs["w_k"], np.float32)
    b_k = np.asarray(inputs["b_k"], np.float32)
    w_v = np.asarray(inputs["w_v"], np.float32)
    b_v = np.asarray(inputs["b_v"], np.float32)
    w_o = np.asarray(inputs["w_o"], np.float32)

    xTb = [np.ascontiguousarray(x[b].T).astype(bf16) for b in range(B)]
    cTb = [np.ascontiguousarray(context[b].T).astype(bf16) for b in range(B)]
    ident = np.eye(128, dtype=np.float32).astype(bf16)
    vones = np.ones((128, NLK * GH), np.float32).astype(bf16)
    maps = []
    for c in range(8):
        b, g = c // 4, c % 4
        hs = slice(256 * g, 256 * (g + 1))
        maps.append({
            "xT": xTb[b],
            "ctxT": cTb[b],
            # scale by 2^-8 (exact in fp): exp scale 32 * score scale 1/8
            "wq": (np.ascontiguousarray(w_q[:, hs]) / 256.0).astype(bf16),
            "wk": np.ascontiguousarray(w_k[:, hs]).astype(bf16),
            "wv": np.ascontiguousarray(w_v[:, hs]).astype(bf16),
            "wo": np.ascontiguousarray(w_o[hs, :]).astype(bf16),
            "bq": np.ascontiguousarray((b_q[hs] / 256.0).reshape(2, 128).T),
            "bk": np.ascontiguousarray(b_k[hs].reshape(2, 128).T),
            "bv": np.broadcast_to(b_v[None, hs], (128, 256)).copy(),
            "ident": ident,
            "vones": vones,
        })
    return maps


def kernel_run(inputs, trace=False, **kw):
    """Run on HW; returns (full_output, BassKernelResults)."""
    b_o = np.asarray(inputs["b_o"], np.float32)
    maps = prep_maps(inputs)
    nc = _get_nc()
    res = bass_utils.run_bass_kernel_spmd(nc, maps, core_ids=list(range(8)),
                                          trace=trace, **kw)
    out = np.empty((B, LQ, D), np.float32)
    for b in range(B):
        acc = res.results[4 * b]["outT"].astype(np.float32)
        acc += res.results[4 * b]["outT2"]
        for g in range(1, 4):
            acc = acc + res.results[4 * b + g]["outT"]
            acc = acc + res.results[4 * b + g]["outT2"]
        out[b] = acc.T + b_o[None, :]
    return out, res


def kernel(**inputs) -> np.ndarray:
    out, _ = kernel_run(inputs)
    return out


# revision 32
# speedup vs baseline: 1.1182x; 1.0101x over previous
"""Cross-attention Trainium2 kernel (nn_CrossAttention, B=2, L=2048, D=1024,
Dctx=768, 16 heads x 64).

Sharding: 8 cores = 2 (batch) x 4 (head-groups of 4 heads). Each core computes
its batch's Q/K/V projections for its 4 heads, attention, and a partial output
projection; the host sums the head-group partials and adds b_o.

v2 design (CoreSim cost model):
- AV computed TRANSPOSED: per (half, head, q-block of 128), accumulate
  out_T[q128, 65] = sum_j ex_j[qb-chunk].T @ [ones|v_j] over the 16 key
  blocks, with ex as the (free-to-reload) stationary operand. Cost-model
  charge is out-free-size (65) per matmul, halving AV cost vs the
  untransposed form. Column 0 carries the softmax denominator.
- Per-qb normalize (reciprocal_approx_fast on a strided d-gather + one
  tensor_scalar_mul), then a PE transpose (identity matmul) back to
  [64, q] and a DVE copy into aT for the output projection.
- exp runs on ACT (scale=32; wq/bq pre-scaled by 1/256 on host so scores
  arrive as u/32) with a few tiles offloaded to 2-inst custom-DVE
  polynomial exp (p3(u/32)^32, rel err ~5e-4) to balance engines.
- ex tiles are bf16 (stationary operand; halves SBUF so a full head-half
  of 16 tiles stays live for the lagged transposed-AV pass).
- Tail split: the final pair's (s2,s3) out-projection contraction is
  emitted as two bf16 partials (outT + outT2) summed on host, so the p0
  half runs a window early instead of serializing in the tail.
"""
import numpy as np

import concourse.bass as bass
import concourse.tile as tile
from concourse import bacc, mybir, bass_utils

# ---- custom DVE exp ops (runtime registration, documented extension path) ---
import concourse.dve_ops as dve_ops
from concourse.dve_ops import DveOp, OPS, CUSTOM_DVE_SPECS, _SUB_OPCODE_FOR_NAME
from concourse.dve_spec import Spec, Src0, C0, C1, C2, One, lower, sq
from concourse.dve_uop import DveOpSpec

_t = Src0
_p3 = One + _t * (C0 + _t * (C1 + _t * C2))


def _ref_exp_a(in0, in1, c0, c1, c2):
    t = in0.astype(np.float32)
    p = 1.0 + t * (np.float32(c0) + t * (np.float32(c1) + t * np.float32(c2)))
    return (p * p).astype(np.float32)


def _ref_exp_b(in0, in1, c0, c1, c2):
    y = in0.astype(np.float32)
    y = y * y
    y = y * y
    y = y * y
    return (y * y).astype(np.float32)


def _register(name, spec):
    if name in _SUB_OPCODE_FOR_NAME:
        return next(o for o in OPS if o.name == name)
    row = dve_ops._CUSTOM_DVE_ROW_BASE + len(OPS)
    assert row < 0x20
    _SUB_OPCODE_FOR_NAME[name] = row
    shas = {}
    for ver in ("v3", "v4"):
        s = DveOpSpec(name=name, opcode=row, uops=lower(spec, ver=ver),
                      rd1_en=False)
        shas[ver] = s.sha(ver)
    op = DveOp(name, spec, subdim=False, uops_sha=shas)
    OPS.append(op)
    CUSTOM_DVE_SPECS[name] = spec
    return op


EXP_A = _register("ANT_EXP_P3SQ1", Spec(body=sq(_p3), reference=_ref_exp_a))
EXP_B = _register("ANT_SQ4", Spec(body=sq(sq(sq(sq(Src0)))), reference=_ref_exp_b))

# minimax-ish p3 for e^t on [-0.27, 0.27] (c0 normalized to 1; the global
# p(0)^32 factor cancels in softmax): c1, c2, c3
EC1, EC2, EC3 = 1.00005423, 0.50272472, 0.16640462

F32R = mybir.dt.float32r
F32 = mybir.dt.float32
BF16 = mybir.dt.bfloat16
EXP = mybir.ActivationFunctionType.Exp

# Problem shape (hardcoded per harness contract)
B, LQ, D = 2, 2048, 1024
DCTX = 768
NH, HD = 16, 64
SCALE = 1.0 / 8.0

GH = 4                # heads per core
VW = HD + 1           # 65: [ones | v] lane per (j, h)
VAW = GH * VW         # 260
KT_Q = D // 128       # 8
KT_C = DCTX // 128    # 6
NLK = LQ // 128       # 16 key blocks
NQB = 8               # q-blocks of 128 per half
HALF = 1024


DEBUG = False


def _build():
    nc = bacc.Bacc("TRN2", target_bir_lowering=False, debug=False,
                   enable_asserts=False, num_devices=8)

    xT_d = nc.dram_tensor("xT", (D, LQ), BF16, kind="ExternalInput").ap()
    cT_d = nc.dram_tensor("ctxT", (DCTX, LQ), BF16, kind="ExternalInput").ap()
    wq_d = nc.dram_tensor("wq", (D, 256), BF16, kind="ExternalInput").ap()
    wk_d = nc.dram_tensor("wk", (DCTX, 256), BF16, kind="ExternalInput").ap()
    wv_d = nc.dram_tensor("wv", (DCTX, 256), BF16, kind="ExternalInput").ap()
    wo_d = nc.dram_tensor("wo", (256, D), BF16, kind="ExternalInput").ap()
    bq_d = nc.dram_tensor("bq", (128, 2), F32, kind="ExternalInput").ap()
    bk_d = nc.dram_tensor("bk", (128, 2), F32, kind="ExternalInput").ap()
    bv_d = nc.dram_tensor("bv", (128, 256), F32, kind="ExternalInput").ap()
    id_d = nc.dram_tensor("ident", (128, 128), BF16, kind="ExternalInput").ap()
    vo_d = nc.dram_tensor("vones", (128, NLK * GH), BF16,
                          kind="ExternalInput").ap()
    out_d = nc.dram_tensor("outT", (D, LQ), BF16, kind="ExternalOutput").ap()
    out2_d = nc.dram_tensor("outT2", (D, LQ), BF16, kind="ExternalOutput").ap()

    with tile.TileContext(nc) as tc:
        with tc.tile_pool(name="w", bufs=1) as wp, \
             tc.tile_pool(name="xt", bufs=2) as xtp, \
             tc.tile_pool(name="ct", bufs=4) as ctp, \
             tc.tile_pool(name="act", bufs=1) as actp, \
             tc.tile_pool(name="expp", bufs=34) as expp, \
             tc.tile_pool(name="tmpp", bufs=2) as tmpp, \
             tc.tile_pool(name="attn", bufs=6) as attnp, \
             tc.tile_pool(name="rdp", bufs=4) as rdp, \
             tc.tile_pool(name="outp", bufs=6) as outp, \
             tc.tile_pool(name="stp", bufs=2, space="PSUM") as stp, \
             tc.tile_pool(name="avp", bufs=1, space="PSUM") as avp, \
             tc.tile_pool(name="scr", bufs=2, space="PSUM") as scr:

            # ---- earliest deps first: K path ----
            wk_t = wp.tile([128, KT_C * 256], BF16, tag="wk")
            nc.gpsimd.dma_start(wk_t[:].rearrange("p (kt m) -> p kt m", m=256),
                                wk_d.rearrange("(kt p) m -> p kt m", p=128))
            bk_t = wp.tile([128, 2], F32, tag="bk")
            nc.scalar.dma_start(bk_t[:], bk_d[:])

            # ---- persistent activation tiles ----
            qT = [actp.tile([128, LQ], F32R, tag=f"qT{p}", name=f"qT{p}")
                  for p in range(2)]
            kT = [actp.tile([128, LQ], F32R, tag=f"kT{p}", name=f"kT{p}")
                  for p in range(2)]
            v_t = actp.tile([128, NLK * VAW], BF16, tag="v")
            aT = [actp.tile([128, LQ], BF16, tag=f"aT{p}", name=f"aT{p}")
                  for p in range(2)]
            id_t = wp.tile([128, 128], BF16, tag="id")

            # warm the ACT exp table during the lead-in DMAs
            warm_t = tmpp.tile([128, 1], F32, tag="warm")
            nc.scalar.activation(warm_t[:], bk_t[:, 0:1], EXP, scale=1.0)

            # ones columns of v_t via one strided DMA
            nc.scalar.dma_start(
                v_t[:].rearrange("p (l w) -> p l w", w=VW)[:, :, 0:1],
                vo_d.rearrange("p (l o) -> p l o", o=1))

            wq_t = wp.tile([128, KT_Q * 256], BF16, tag="wq")
            bq_t = wp.tile([128, 2], F32, tag="bq")
            wv_t = wp.tile([128, KT_C * 256], BF16, tag="wv")
            bv_t = wp.tile([128, 256], F32, tag="bv")
            wo_t = wp.tile([128, 2 * D], BF16, tag="wo")
            ct_tiles = {}
            xt_tiles = {}

            def ct_dma(s, eng=None):
                eng = eng or nc.sync
                t = ctp.tile([128, KT_C * 512], BF16, tag="ct")
                tv = t[:].rearrange("p (kt q) -> p kt q", q=512)
                cv = cT_d.rearrange("(kt p) q -> p kt q",
                                    p=128)[:, :, 512 * s:512 * (s + 1)]
                for kk in range(3):
                    eng.dma_start(tv[:, 2 * kk:2 * kk + 2, :],
                                  cv[:, 2 * kk:2 * kk + 2, :])
                ct_tiles[s] = t

            def k_proj(s, p):
                if s not in ct_tiles:
                    ct_dma(s)
                t = ct_tiles[s]
                ps = scr.tile([128, 512], F32, tag="u")
                for kt in range(KT_C):
                    nc.tensor.matmul(
                        ps[:], wk_t[:, 256 * kt + 128 * p:256 * kt + 128 * (p + 1)],
                        t[:, 512 * kt:512 * (kt + 1)],
                        start=(kt == 0), stop=(kt == KT_C - 1))
                nc.vector.tensor_scalar_add(
                    kT[p][:, 512 * s:512 * (s + 1)], ps[:], bk_t[:, p:p + 1])

            def xt_dma(s, eng=None):
                eng = eng or nc.sync
                t = xtp.tile([128, KT_Q * 512], BF16, tag="xt")
                tv = t[:].rearrange("p (kt q) -> p kt q", q=512)
                xv = xT_d.rearrange("(kt p) q -> p kt q",
                                    p=128)[:, :, 512 * s:512 * (s + 1)]
                for kk in range(2):
                    eng.dma_start(tv[:, 4 * kk:4 * kk + 4, :],
                                  xv[:, 4 * kk:4 * kk + 4, :])
                xt_tiles[s] = t

            def q_proj(s, p):
                if s not in xt_tiles:
                    xt_dma(s)
                t = xt_tiles[s]
                ps = scr.tile([128, 512], F32, tag="u")
                for kt in range(KT_Q):
                    nc.tensor.matmul(
                        ps[:], wq_t[:, 256 * kt + 128 * p:256 * kt + 128 * (p + 1)],
                        t[:, 512 * kt:512 * (kt + 1)],
                        start=(kt == 0), stop=(kt == KT_Q - 1))
                nc.vector.tensor_scalar_add(
                    qT[p][:, 512 * s:512 * (s + 1)], ps[:], bq_t[:, p:p + 1])

            def v_chunk(j):
                ps = scr.tile([128, 256], F32, tag="u")
                s, jj = j // 4, j % 4
                for kt in range(KT_C):
                    nc.tensor.matmul(
                        ps[:],
                        ct_tiles[s][:, 512 * kt + 128 * jj:512 * kt + 128 * (jj + 1)],
                        wv_t[:, 256 * kt:256 * (kt + 1)],
                        start=(kt == 0), stop=(kt == KT_C - 1))
                vv = v_t[:, VAW * j:VAW * (j + 1)].rearrange(
                    "p (h w) -> p h w", w=VW)[:, :, 1:VW]
                nc.vector.tensor_add(
                    vv, ps[:].rearrange("p (h w) -> p h w", w=HD),
                    bv_t[:].rearrange("p (h w) -> p h w", w=HD))

            # ---- per-(half, head) attention state ----
            avt = [avp.tile([128, 512], F32, tag=f"avt{i}", name=f"avt{i}")
                   for i in range(2)]

            def avt_run(ctx, qb, js=None):
                """Transposed-AV accumulation MMs (j subset) for one head."""
                exs, h_ = ctx["exs"], ctx["h"]
                ps = avt[qb // 4]
                off = VW * (qb % 4)
                for j in (js if js is not None else range(NLK)):
                    nc.tensor.matmul(
                        ps[:, off:off + VW],
                        exs[j][:, 128 * qb:128 * (qb + 1)],
                        v_t[:, VAW * j + VW * h_:VAW * j + VW * (h_ + 1)],
                        start=(j == 0), stop=(j == NLK - 1))

            def avt_recip(ctx, bank):
                """1/d for the 4 qb blocks of one avt bank."""
                rd = rdp.tile([128, 4], F32, tag=f"rd{bank}")
                dg = avt[bank][:, 0:4 * VW:VW]
                nc.vector.reciprocal_approx_fast(rd[:], dg)
                ctx[f"rd{bank}"] = rd

            def avt_norm(ctx, bank):
                """normalize a whole avt bank -> attn_sb bf16 [128q, 4, 64]."""
                rd = ctx[f"rd{bank}"]
                at = attnp.tile([128, 4 * HD], BF16, tag="at")
                src = avt[bank][:, 0:4 * VW].rearrange(
                    "p (b w) -> p b w", w=VW)[:, :, 1:VW]
                rdb = rd[:].rearrange("p (b o) -> p b o", o=1).broadcast_to(
                    [128, 4, HD])
                nc.vector.tensor_mul(
                    at[:].rearrange("p (b w) -> p b w", w=HD), src, rdb)
                ctx[f"at{bank}"] = at

            def avt_tr(ctx, qb):
                """PE-transpose attn qb into the bank's tp psum tile."""
                half_, h_ = ctx["half"], ctx["h"]
                m_ = h_ % 2
                bank = qb // 4
                at = ctx[f"at{bank}"]
                if qb % 4 == 0:
                    tpn = scr.tile([128, 4 * 128], BF16, tag="u")
                    ctx[f"tp{bank}"] = tpn
                tp = ctx[f"tp{bank}"]
                nc.tensor.matmul(
                    tp[64 * m_:64 * (m_ + 1), 128 * (qb % 4):128 * (qb % 4 + 1)],
                    at[:, HD * (qb % 4):HD * (qb % 4 + 1)], id_t[:],
                    is_transpose=True)

            def avt_cp(ctx, bank):
                """copy a bank of transposed attn into aT."""
                half_, h_ = ctx["half"], ctx["h"]
                p_, m_ = h_ // 2, h_ % 2
                tp = ctx.pop(f"tp{bank}")
                nc.vector.tensor_copy(
                    aT[p_][64 * m_:64 * (m_ + 1),
                           HALF * half_ + 512 * bank:HALF * half_ + 512 * (bank + 1)],
                    tp[64 * m_:64 * (m_ + 1), :])

            def out_unit(mo, s, p_, on_act=False):
                """out-proj partial unit for pair p_: one MM + copy + DMA."""
                ps = scr.tile([128, 512], F32, tag="u")
                nc.tensor.matmul(
                    ps[:], wo_t[:, D * p_ + 128 * mo:D * p_ + 128 * (mo + 1)],
                    aT[p_][:, 512 * s:512 * (s + 1)],
                    start=True, stop=True)
                ot = outp.tile([128, 512], BF16, tag="out")
                if on_act:
                    nc.scalar.copy(ot[:], ps[:])
                else:
                    nc.vector.tensor_copy(ot[:], ps[:])
                dst = out_d if p_ == 0 else out2_d
                nc.gpsimd.dma_start(
                    dst[128 * mo:128 * (mo + 1), 512 * s:512 * (s + 1)], ot[:])

            # ---- lead-in: DMAs spread across engine queues ----
            nc.scalar.dma_start(wq_t[:].rearrange("p (kt m) -> p kt m", m=256),
                                wq_d.rearrange("(kt p) m -> p kt m", p=128))
            xt_dma(0)                       # SP
            ct_dma(0, nc.gpsimd)            # Pool (behind wk)
            nc.scalar.dma_start(bq_t[:], bq_d[:])
            xt_dma(1)                       # SP
            ct_dma(1, nc.gpsimd)
            nc.scalar.dma_start(id_t[:], id_d[:])
            nc.gpsimd.dma_start(wv_t[:].rearrange("p (kt m) -> p kt m", m=256),
                                wv_d.rearrange("(kt p) m -> p kt m", p=128))
            nc.scalar.dma_start(bv_t[:], bv_d[:])
            nc.gpsimd.dma_start(wo_t[:].rearrange("p (p2 m) -> p p2 m", m=1024),
                                wo_d.rearrange("(p2 p) m -> p p2 m", p=128))
            k_proj(0, 0)
            q_proj(0, 0)
            q_proj(1, 0)

            # ---- drip worklists per window (list of (pos, 0-arg callable)) --
            def W(fn, *a):
                return lambda: fn(*a)

            def retire_steps(ctx, t0, dt):
                """AV_T + recip + norm + transpose steps for a finished head,
                spread from emission position t0 with spacing dt."""
                st = []
                t = t0
                for b in range(2):
                    for qq in range(4):
                        st.append((t, W(avt_run, ctx, 4 * b + qq)))
                        t += dt
                    st.append((t, W(avt_recip, ctx, b)))
                    t += dt / 4
                for bank in range(2):
                    st.append((t, W(avt_norm, ctx, bank)))
                    t += dt / 3
                    for qq in range(4):
                        st.append((t, W(avt_tr, ctx, 4 * bank + qq)))
                        t += dt / 3
                    st.append((t, W(avt_cp, ctx, bank)))
                    t += dt / 3
                return st

            # DVE-offloaded exp tiles per window
            DVE_JS = set()

            def emit_exp(st, j, on_dve):
                ex = expp.tile([128, HALF], BF16, tag="expS")
                if on_dve:
                    stg = tmpp.tile([128, HALF], F32, tag="stg")
                    nc.scalar.copy(stg[:], st[:])
                    tmp = tmpp.tile([128, HALF], F32, tag="tmp")
                    nc.vector._custom_dve(EXP_A, out=tmp[:], in0=stg[:],
                                          s0=EC1, s1=EC2, imm2=EC3)
                    nc.vector._custom_dve(EXP_B, out=ex[:], in0=tmp[:])
                else:
                    nc.scalar.activation(ex[:], st[:], EXP, scale=32.0)
                return ex

            # static drips (beyond the retire pipeline), per window
            wl = [[] for _ in range(9)]
            wl[0] = [
                (1, W(ct_dma, 2)), (2, W(k_proj, 1, 0)), (3, W(v_chunk, 0)),
                (4, W(v_chunk, 1)), (4.5, W(ct_dma, 3)), (5, W(k_proj, 2, 0)),
                (6, W(v_chunk, 2)), (7, W(v_chunk, 3)), (8, W(v_chunk, 4)),
                (9, W(k_proj, 3, 0)), (10, W(v_chunk, 5)), (11, W(v_chunk, 6)),
                (12, W(v_chunk, 7)), (13, W(v_chunk, 8)), (14, W(v_chunk, 9)),
            ]
            wl[1] = [
                (0, W(v_chunk, 10)), (1, W(v_chunk, 11)), (2, W(v_chunk, 12)),
                (3, W(v_chunk, 13)), (4, W(v_chunk, 14)), (5, W(v_chunk, 15)),
                (5.5, W(k_proj, 0, 1)), (8, W(q_proj, 0, 1)),
                (11, W(q_proj, 1, 1)),
            ]
            wl[2] = [(1, W(k_proj, 1, 1)),
                     (6, W(k_proj, 2, 1)), (10, W(k_proj, 3, 1))]
            wl[3] = ([(2, W(q_proj, 2, 0)), (5, W(q_proj, 3, 0))]
                     + [(8 + m, W(out_unit, m, m % 2, 0)) for m in range(8)])
            wl[4] = ([(2, W(q_proj, 2, 1)), (5, W(q_proj, 3, 1))]
                     + [(8 + m, W(out_unit, m, 1 - m % 2, 0)) for m in range(8)])
            wl[5] = ([(4 + m, W(out_unit, m, m % 2, 1)) for m in range(8)]
                     + [(12.5 + m / 4, W(out_unit, m, 1 - m % 2, 1))
                        for m in range(4)])
            wl[6] = ([(4 + m, W(out_unit, m, 1 - m % 2, 1)) for m in range(4, 8)]
                     + [(9 + m, W(out_unit, m, 2 + m % 2, 0)) for m in range(4)])
            wl[7] = ([(2 + m, W(out_unit, m, 2 + m % 2, 0)) for m in range(4, 8)]
                     + [(8 + m / 2, W(out_unit, m, 3 - m % 2, 0))
                        for m in range(8)])
            wl[8] = []

            # ---- main windows ----
            ctx_prev = None
            ctx_last = {}
            for w in range(8):
                half, h = w // 4, w % 4
                p, m = h // 2, h % 2
                r0 = 64 * m
                work = list(wl[w])
                if ctx_prev is not None:
                    # w1: v_chunks land j0-5, retire after; else spread early
                    work += retire_steps(ctx_prev, 6.0 if w == 1 else 0.5, 0.6)
                if w == 7:
                    ctx_last.update(half=half, h=h)
                    # drip the last head's AV_T (j0..11) through this window
                    work += [(13 + qb * 0.12, W(avt_run, ctx_last, qb,
                                                range(12)))
                             for qb in range(4)]
                    work += [(14 + qb * 0.12, W(avt_run, ctx_last, 4 + qb,
                                                range(12)))
                             for qb in range(4)]
                work.sort(key=lambda t: t[0])
                wi = 0
                exs = ctx_last.setdefault("exs", []) if w == 7 else []
                for j in range(NLK):
                    while wi < len(work) and work[wi][0] <= j:
                        work[wi][1]()
                        wi += 1
                    st = stp.tile([128, HALF], F32, tag="st")
                    for n in range(2):
                        nc.tensor.matmul(
                            st[:, 512 * n:512 * (n + 1)],
                            kT[p][r0:r0 + 64, 128 * j:128 * (j + 1)],
                            qT[p][r0:r0 + 64,
                                  HALF * half + 512 * n:HALF * half + 512 * (n + 1)],
                            start=True, stop=True)
                    exs.append(emit_exp(st, j, j in DVE_JS))
                while wi < len(work):
                    work[wi][1]()
                    wi += 1
                ctx_prev = {"exs": exs, "half": half, "h": h}

            # ---- tail: finish h3-half1 AV_T, retire, p1 units per bank ----
            for bank in range(2):
                for qb in range(4):
                    avt_run(ctx_last, 4 * bank + qb, range(12, NLK))
                avt_recip(ctx_last, bank)
                avt_norm(ctx_last, bank)
                for qb in range(4):
                    avt_tr(ctx_last, 4 * bank + qb)
                avt_cp(ctx_last, bank)
                for mo in range(8):
                    out_unit(mo, 2 + bank, 1, on_act=(mo % 2 == 0))

            if DEBUG:
                dbg_a = nc.dram_tensor("dbg_aT0", (128, LQ), F32,
                                       kind="ExternalOutput").ap()
                dbg_q = nc.dram_tensor("dbg_qT0", (128, LQ), F32,
                                       kind="ExternalOutput").ap()
                dbg_k = nc.dram_tensor("dbg_kT0", (128, LQ), F32,
                                       kind="ExternalOutput").ap()
                dbg_a1 = nc.dram_tensor("dbg_aT1", (128, LQ), F32,
                                        kind="ExternalOutput").ap()
                for nm, dst, src in (("a", dbg_a, aT[0]), ("q", dbg_q, qT[0]),
                                     ("k", dbg_k, kT[0]), ("a1", dbg_a1, aT[1])):
                    for c in range(4):
                        t = outp.tile([128, 512], F32, tag="dbg")
                        nc.vector.tensor_copy(t[:], src[:, 512 * c:512 * (c + 1)])
                        nc.gpsimd.dma_start(dst[:, 512 * c:512 * (c + 1)], t[:])

    nc.compile()
    return nc


_NC_CACHE = []


def _get_nc():
    if not _NC_CACHE:
        _NC_CACHE.append(_build())
    return _NC_CACHE[0]


OUT_NAME = "outT"


def prep_maps(inputs):
    """Host-side prep: per-core input tensor maps."""
    import ml_dtypes
    bf16 = ml_dtypes.bfloat16
    x = np.asarray(inputs["x"], np.float32)
    context = np.asarray(inputs["context"], np.float32)
    w_q = np.asarray(inputs["w_q"], np.float32)
    b_q = np.asarray(inputs["b_q"], np.float32)
    w_k = np.asarray(inputs["w_k"], np.float32)
    b_k = np.asarray(inputs["b_k"], np.float32)
    w_v = np.asarray(inputs["w_v"], np.float32)
    b_v = np.asarray(inputs["b_v"], np.float32)
    w_o = np.asarray(inputs["w_o"], np.float32)

    xTb = [np.ascontiguousarray(x[b].T).astype(bf16) for b in range(B)]
    cTb = [np.ascontiguousarray(context[b].T).astype(bf16) for b in range(B)]
    ident = np.eye(128, dtype=np.float32).astype(bf16)
    vones = np.ones((128, NLK * GH), np.float32).astype(bf16)
    maps = []
    for c in range(8):
        b, g = c // 4, c % 4
        hs = slice(256 * g, 256 * (g + 1))
        maps.append({
            "xT": xTb[b],
            "ctxT": cTb[b],
            # scale by 2^-8 (exact in fp): exp scale 32 * score scale 1/8
            "wq": (np.ascontiguousarray(w_q[:, hs]) / 256.0).astype(bf16),
            "wk": np.ascontiguousarray(w_k[:, hs]).astype(bf16),
            "wv": np.ascontiguousarray(w_v[:, hs]).astype(bf16),
            "wo": np.ascontiguousarray(w_o[hs, :]).astype(bf16),
            "bq": np.ascontiguousarray((b_q[hs] / 256.0).reshape(2, 128).T),
            "bk": np.ascontiguousarray(b_k[hs].reshape(2, 128).T),
            "bv": np.broadcast_to(b_v[None, hs], (128, 256)).copy(),
            "ident": ident,
            "vones": vones,
        })
    return maps


def kernel_run(inputs, trace=False, **kw):
    """Run on HW; returns (full_output, BassKernelResults)."""
    b_o = np.asarray(inputs["b_o"], np.float32)
    maps = prep_maps(inputs)
    nc = _get_nc()
    res = bass_utils.run_bass_kernel_spmd(nc, maps, core_ids=list(range(8)),
                                          trace=trace, **kw)
    out = np.empty((B, LQ, D), np.float32)
    for b in range(B):
        acc = res.results[4 * b]["outT"].astype(np.float32)
        acc += res.results[4 * b]["outT2"]
        for g in range(1, 4):
            acc = acc + res.results[4 * b + g]["outT"]
            acc = acc + res.results[4 * b + g]["outT2"]
        out[b] = acc.T + b_o[None, :]
    return out, res


def kernel(**inputs) -> np.ndarray:
    out, _ = kernel_run(inputs)
    return out
